# revision 84
# baseline (speedup 1.0000x reference)
"""Trainium2 Bass kernel for nn_AttenModule (B=64, N=1024, M=80, C1=288, D=256).

Math notes (derived from the reference):
  score[b,n,m] = (oa@w_o)[b,n] + (lang@w_l)[b,m] + ba, softmax over m.
  The (oa@w_o)[b,n] and ba terms are constant along m, so they cancel in the
  softmax -> att[b,n,:] == softmax_m(mask(lang[b]@w_l)) is independent of n,
  and att_feat[b,:] = sum_m att[b,m]*lang[b,m,:] is a per-batch vector.
  Hence the entire W1/W2/w_o branch is dead.

  out = (v@Ws)/max(||v||,eps) + bs with v = relu(osc * af[b]) is
  scale-invariant in v (relu commutes with positive scales), so the softmax
  denominator only needs enough accuracy to keep fp16 ranges in check.

  Remaining per-row work (row = (b,n)):
    osc = relu(x @ W3 + b3) @ W4 + b4            # x = object_feat row (288,)
    v   = relu(osc * af[b])                      # af[b] = softmax(lang@w_l) @ lang
    out = (v @ Ws) / sqrt(||v||^2 + 1e-24) + bs

Device layout: feature-on-partition (transposed activations).  Per core
(8 cores, data-parallel over B): 8 batches = 8192 rows, row-tiles of 512.
All matmuls run in fp16 (11-bit mantissa, 1 cycle/row); PSUM accumulates
fp32.  Biases and the attention scale vector stay fp32 (applied via the
ACT engine's per-partition scale/bias).

Scheduling notes (v2):
  - Software pipelining: tile t+1's L1 matmuls are emitted before tile t's
    h/L2, so the PE runs next tile's L1 while DVE turns around h0/h1 and
    the h0 latency leaves the critical cycle (steady period ~2.7us/tile,
    PE-bound at ~11 matmul slots).
  - Engine split per tile: DVE h0/h1 (+merged xq at 1-tile lag, pair
    out-mul); ACT xv0/xv1 (+pair rsqrt); ACT keeps ~0.8us/tile of slack to
    drain its pipe-fill backlog (exp + early xv bursts).
  - Epilogues are batched per 2 tiles ("pair"): tile 2p's dot/ss go to PSUM
    partitions 0-31/64-95 of one bank (32-wide PE column groups), tile
    2p+1's to 32-63/96-127.  One ACT rsqrt over partitions 64..96 and one
    DVE multiply over 0..32 then serve both tiles (ACT/DVE cost is
    partition-count independent).  Matvecs trail the main loop by MV_LAG
    tiles so the ACT-produced xv/xq are always ready.
  - attention is split: scores+exp before the loop (PE/ACT), the
    mask/denominator/af chain inside tile 0 (where its PE matmuls fill the
    exp->em dependency bubble); psum tiles borrow the po0/po1 tags so the
    ph0/ph1 double-buffer rotation stays clean across tiles 0-2.
  - DMA: the 16 engines round-robin all non-empty HWDGE queues with no
    priorities, one queue sustains only ~110-180 GB/s, each dma_start costs
    ~700ns of serial issue time on its engine, and reused semaphores force
    issue-order waits.  So: the fill rides one deadline-ordered FIFO on
    sync (weights, batch-0 x in halves, langm, then batches 1-3), langt2
    alone on gpsimd, and batches 4-7 are emitted mid-loop where the xp-pool
    WAR gate self-paces them ~3 tiles ahead of use.
  - The HAM clock gate holds the PE at 1.2 GHz until ~3.4us of sustained
    activity and re-arms on any >~0.8us idle (costing ~10us of half clock):
    N_WARM dummy matmuls cover the first x/weight DMAs, and the mv-tagged
    bridge dummies WAR-wait on the exp read of ps_sl, which lands them
    exactly in the exp->em PE hole.
"""

import numpy as np

import concourse.bacc as bacc
import concourse.tile as tile
from concourse import mybir
from concourse.bass_utils import run_bass_kernel_spmd

B, N, M = 64, 1024, 80
C1, D = 288, 256
NCORES = 8
BPC = B // NCORES          # batches per core
R = BPC * N                # rows per core
TILE = 512
NT = R // TILE             # row tiles per core
NPAIR = NT // 2
F32 = mybir.dt.float32
F16 = mybir.dt.float16

N_WARM = 8                 # initial PE warm-up matmuls (HAM clock gate)
N_BRIDGE = 3               # bridge warmups over the attention exp->denom gap
XQ_LAG = 1                 # tiles of lag for the merged xq = xv*xv DVE op
MV_LAG = 3                 # tiles of lag for the matvec + epilogue


def _build_nc():
    nc = bacc.Bacc("TRN2", target_bir_lowering=False, debug=False)

    # xt rows 0..287 = x^T; rows 288..319 duplicate rows 256..287 so the two
    # K=32 tail matmuls (one per out-chunk) can run in concurrent PE row-groups
    xt_d = nc.dram_tensor("xt", [C1 + 32, R], F16, kind="ExternalInput").ap()
    langm_d = nc.dram_tensor("langm", [M, BPC, D], F16, kind="ExternalInput").ap()
    # langt2 pre-arranged as [128, 2, BPC*M] on the host
    langt2_d = nc.dram_tensor("langt2", [128, 2 * BPC * M], F16, kind="ExternalInput").ap()
    # w3/w4 pre-arranged as [128, 2, D] (chunk-major)
    w3_d = nc.dram_tensor("w3", [128, 2 * D], F16, kind="ExternalInput").ap()
    # w3c2: rows 0-31 = W3[256:288, 0:128], rows 32-63 = W3[256:288, 128:256]
    w3c_d = nc.dram_tensor("w3c", [64, 128], F16, kind="ExternalInput").ap()
    w4_d = nc.dram_tensor("w4", [128, 2 * D], F16, kind="ExternalInput").ap()
    # packed fp32 consts: cols [b3(2) | b4(2) | bs(1) | maskt(8, rows 0-79) |
    #                           b4rep(16 = b4 chunk-major replicated per batch)]
    cstf_d = nc.dram_tensor("cstf", [128, 29], F32, kind="ExternalInput").ap()
    # packed fp16 consts: cols [ws(2) | wl(2)]
    csth_d = nc.dram_tensor("csth", [128, 4], F16, kind="ExternalInput").ap()
    out_d = nc.dram_tensor("out", [1, R], F32, kind="ExternalOutput").ap()

    AF = mybir.ActivationFunctionType

    with tile.TileContext(nc) as tc:
        with tc.tile_pool(name="const", bufs=1) as cp:
            # csth/cstf at the HEAD of the sync queue: tiny, and csth feeds
            # the ws32 copies + attention scores
            csth = cp.tile([128, 4], F16)
            nc.sync.dma_start(out=csth, in_=csth_d)
            cstf = cp.tile([128, 29], F32)
            nc.sync.dma_start(out=cstf, in_=cstf_d)
            w3t = cp.tile([128, 2, D], F16)
            w3c2 = cp.tile([64, 128], F16)
            langt2 = cp.tile([128, 2, BPC, M], F16)
            w4t = cp.tile([128, 2, D], F16)
            langm = cp.tile([M, BPC, D], F16)
            wss = csth[:, 0:2]
            wls = csth[:, 2:4]
            b3s = cstf[:, 0:2]
            b4s = cstf[:, 2:4]
            maskt = cstf[0:M, 5:13]
            b4rep = cstf[:, 13:29].rearrange("p (c b) -> p c b", c=2)
            ones_m = cp.tile([M, 1], F16)
            nc.vector.memset(ones_m, 1.0)
            ones_1x128 = cp.tile([1, 128], F16)
            nc.vector.memset(ones_1x128, 1.0)
            ones_128 = cp.tile([128, 1], F16)
            nc.vector.memset(ones_128, 1.0)
            # 32-wide stationaries for the matvecs (col 0 live, rest zero):
            # each dot/ss matmul then fills a whole 32-partition PE column
            # group, so every psum partition the batched epilogue reads is
            # written (same PE cost -- it scales with the moving free size)
            ws32 = cp.tile([128, 2, 32], F16)
            nc.vector.memset(ws32, 0.0)
            nc.vector.tensor_copy(ws32[:, 0, 0:1], wss[:, 0:1])
            nc.vector.tensor_copy(ws32[:, 1, 0:1], wss[:, 1:2])
            ones32 = cp.tile([128, 32], F16)
            nc.vector.memset(ones32, 0.0)
            nc.vector.memset(ones32[:, 0:1], 1.0)
            eps_sb = cp.tile([33, 1], F32)
            nc.vector.memset(eps_sb, 1e-24)
            # PE warm-up: the HAM clock gate keeps the PE at 1.2 GHz until it
            # sees ~3.4us of sustained activity, and re-throttles (costing
            # ~10us of half-speed) if the PE goes idle again.
            warm = cp.tile([128, TILE], F16)
            nc.gpsimd.memset(warm, 0.0)

            # ---------- main loop (attention interleaved into it) ----------
            with (
                # x tiles use bufs=3 (per-tag): the WAR gate then self-paces
                # batch bb's DMA ~3 tiles ahead of its first use
                tc.tile_pool(name="xt", bufs=3) as xp,
                tc.tile_pool(name="work", bufs=4) as wp,
                tc.tile_pool(name="ep", bufs=3) as epp,
                tc.tile_pool(name="ph0p", bufs=2, space="PSUM") as php0,
                tc.tile_pool(name="ph1p", bufs=2, space="PSUM") as php1,
                tc.tile_pool(name="po0p", bufs=2, space="PSUM") as pop0,
                tc.tile_pool(name="po1p", bufs=1, space="PSUM") as pop1,
                tc.tile_pool(name="pmv", bufs=1, space="PSUM") as pmv,
            ):
                # out2[0, p, :] = tile 2p's outputs, out2[32, p, :] = tile 2p+1's
                out2 = cp.tile([33, NPAIR, TILE], F32)
                # DRAM view [1, 2, NPAIR, TILE]: index [o, k, p, r] = tile 2p+k
                outv = out_d.rearrange("o (p k r) -> o k p r", k=2, r=TILE)
                xtv = xt_d.rearrange("c (bb r) -> c bb r", bb=BPC)

                e_sb = cp.tile([M, BPC], F32)
                em_sb = cp.tile([M, BPC], F16)
                rd32 = cp.tile([1, BPC], F32)
                rdf = cp.tile([1, BPC], F16)
                af = cp.tile([128, 2, BPC], F32)
                b4af = cp.tile([128, 2, BPC], F32)

                def emit_attention_a():
                    # --- attention part A: scores, exp, mask.  The denom
                    # matmul lives in part B: here it would block the PE
                    # queue on the exp->em latency right when tile 0's L1
                    # could run.
                    ps_sl = pmv.tile([M, BPC], F32, tag="mv")
                    for b in range(BPC):
                        for c in range(2):
                            nc.tensor.matmul(
                                ps_sl[:, b : b + 1],
                                langt2[:, c, b, :],
                                wls[:, c : c + 1],
                                start=(c == 0),
                                stop=(c == 1),
                            )
                    nc.scalar.activation(e_sb, ps_sl, AF.Exp)

                def emit_attention_b():
                    # em (exp * mask) lives here, not in part A: on the
                    # in-order DVE queue it would otherwise block tile 0's
                    # h0 behind the exp latency
                    nc.vector.tensor_mul(em_sb, e_sb, maskt)
                    # denom per batch + reciprocal (the per-batch scale cancels
                    # in the output; it only keeps fp16 magnitudes in range)
                    ps_dn = pmv.tile([1, BPC], F32, tag="mv")
                    nc.tensor.matmul(ps_dn, ones_m, em_sb, start=True, stop=True)
                    nc.vector.reciprocal(rd32, ps_dn)
                    nc.vector.tensor_copy(rdf, rd32)
                    # --- attention part B: af matmuls + scaling.  Emitted
                    # after tile 0's L1 so its PE/DVE work fills the
                    # exp->em dependency bubble instead of stalling the head.
                    # psum tiles borrow the po0/po1 tags: they are dead by the
                    # time tile 0's L2 needs the banks, and ph0/ph1 rotation
                    # stays clean for tiles 1..2.
                    ps_rdb = pop0.tile([128, BPC], F32, tag="po0")
                    nc.tensor.matmul(ps_rdb, ones_1x128, rdf, start=True, stop=True)
                    rdb = cp.tile([128, BPC], F32)
                    nc.vector.tensor_copy(rdb, ps_rdb)
                    ps_af = pop1.tile([128, 2, BPC], F32, tag="po1")
                    for b in range(BPC):
                        for c in range(2):
                            nc.tensor.matmul(
                                ps_af[:, c, b : b + 1],
                                langm[:, b, c * 128 : (c + 1) * 128],
                                em_sb[:, b : b + 1],
                                start=True,
                                stop=True,
                            )
                    for c in range(2):
                        nc.vector.tensor_mul(af[:, c, :], ps_af[:, c, :], rdb)
                    nc.vector.tensor_mul(b4af, af, b4rep)

                mv_c1_pending = []

                def _mv_group(pr, c, xsa, xqa, xsb, xqb):
                    # one K-chunk of BOTH tiles' matvecs as a 4-way
                    # concurrent PE column-group issue (measured: 4 groups
                    # with different moving streams issue within ~16ns and
                    # overlap fully -> 1 slot instead of 2):
                    #   tile 2p:   dot -> partitions 0-31,  ss -> 64-95
                    #   tile 2p+1: dot -> partitions 32-63, ss -> 96-127
                    # skip_group_check: the sim's zero-region tracker maps
                    # partition offsets to overlapping phantom rows for
                    # 32-wide column-group outputs; the real groups are
                    # partition-disjoint.
                    st = {"start": c == 0, "stop": c == 1,
                          "skip_group_check": True}
                    nc.tensor.matmul(
                        pr[0:32, :], ws32[:, c, :], xsa[c],
                        tile_position=(0, 0), **st,
                    )
                    nc.tensor.matmul(
                        pr[32:64, :], ws32[:, c, :], xsb[c],
                        tile_position=(0, 32), **st,
                    )
                    nc.tensor.matmul(
                        pr[64:96, :], ones32, xqa[c],
                        tile_position=(0, 64), **st,
                    )
                    nc.tensor.matmul(
                        pr[96:128, :], ones32, xqb[c],
                        tile_position=(0, 96), **st,
                    )

                def emit_mv_c0(a, b):
                    # first K-chunk in this iteration; second chunk + rsqrt
                    # deferred to the next -> every iteration carries exactly
                    # one 4-way mv slot (10 PE slots flat)
                    ta, xsa, xqa = a
                    tb, xsb, xqb = b
                    p = ta // 2
                    pr = pmv.tile([128, TILE], F32, tag="mv", name=f"pair{p}")
                    _mv_group(pr, 0, xsa, xqa, xsb, xqb)
                    mv_c1_pending.append((p, pr, xsa, xqa, xsb, xqb))

                def emit_mv_c1():
                    p, pr, xsa, xqa, xsb, xqb = mv_c1_pending.pop(0)
                    _mv_group(pr, 1, xsa, xqa, xsb, xqb)
                    if True:
                        # batched epilogue for the pair: one rsqrt over
                        # partitions 64..96 (both ss rows; ACT cost is
                        # partition-count independent), one multiply over
                        # partitions 0..32 (both dot rows).  Unwritten
                        # partitions in between produce garbage lanes that
                        # are never stored.  The multiply is deferred to the
                        # FRONT of the next iteration's DVE queue: it balances
                        # the per-iteration DVE load (which otherwise exceeds
                        # the PE period on epilogue iterations) while still
                        # completing before the pair bank's WAR reuse.
                        rc = epp.tile([33, TILE], F32, tag="rc")
                        inst = nc.scalar.activation(
                            rc, pr[64:97, :], AF.Sqrt, bias=eps_sb
                        )
                        inst.ins.func = AF.Rsqrt
                        epi_pending.append((p, pr, rc))

                pending = []
                xq1_pending = []
                epi_pending = []

                def emit_epilogue():
                    p, pr, rc = epi_pending.pop(0)
                    nc.vector.tensor_mul(out2[:, p, :], pr[0:33, :], rc)
                    if p == NPAIR // 2 - 1:
                        # first half of the output: overlap store
                        nc.sync.dma_start(
                            out=outv[:, 0, 0 : NPAIR // 2, :],
                            in_=out2[0:1, 0 : NPAIR // 2, :],
                        )
                        nc.sync.dma_start(
                            out=outv[:, 1, 0 : NPAIR // 2, :],
                            in_=out2[32:33, 0 : NPAIR // 2, :],
                        )
                xtv2 = xt_d[0:256, :].rearrange(
                    "(c p) (bb r) -> p c bb r", c=2, bb=BPC
                )

                # --- batch-0/1 loads.  Queue priorities (Q0=gpsimd,
                # Q10=scalar, Q1=sync; the 16 DMA engines round-robin across
                # queues, so the critical fill tensors must not sit behind
                # bulk x):
                #   gpsimd: langt2 FIRST (gates attention A), x first halves
                #           (tile 0's L1), then x second halves (tile 1)
                #   scalar: consts; batch-1 x is issued AFTER attention A's
                #           exp in the in-order scalar stream, so it starts
                #           flowing only once the critical fill has landed
                #   sync:   w3t (gates L1), langm (attention B), w4t (L2),
                #           then the bb>=2 batch loads (throttled by xp bufs)
                x01_0 = xp.tile([128, 2, N], F16, tag="x01", name="x01_0", bufs=3)
                x2_0 = xp.tile([64, N], F16, tag="x2", name="x2_0", bufs=3)
                x01_1 = xp.tile([128, 2, N], F16, tag="x01", name="x01_1", bufs=3)
                x2_1 = xp.tile([64, N], F16, tag="x2", name="x2_1", bufs=3)
                # Fill transfers: each dma_start costs ~700ns of SERIAL issue
                # time on its engine, and 17 descriptors on one queue starve
                # the DMA engines between issues (measured 103-181 GB/s dips
                # vs 349 peak).  Spread the ISSUE load over three engines,
                # each queue ordered by its own deadlines.
                nc.sync.dma_start(out=w3t, in_=w3_d.rearrange("p (c d) -> p c d", c=2))
                nc.scalar.dma_start(out=langm, in_=langm_d)
                nc.gpsimd.dma_start(out=w3c2, in_=w3c_d)
                nc.sync.dma_start(out=x01_0[:, 0, 0:TILE], in_=xtv2[:, 0, 0, 0:TILE])
                nc.gpsimd.dma_start(out=x01_0[:, 0, TILE:N], in_=xtv2[:, 0, 0, TILE:N])
                nc.scalar.dma_start(
                    out=w4t, in_=w4_d.rearrange("p (c d) -> p c d", c=2)
                )
                nc.sync.dma_start(out=x01_0[:, 1, 0:TILE], in_=xtv2[:, 1, 0, 0:TILE])
                nc.gpsimd.dma_start(out=x01_0[:, 1, TILE:N], in_=xtv2[:, 1, 0, TILE:N])
                nc.sync.dma_start(out=x2_0[:, 0:TILE], in_=xtv[256:320, 0, 0:TILE])
                nc.gpsimd.dma_start(out=x2_0[:, TILE:N], in_=xtv[256:320, 0, TILE:N])
                nc.gpsimd.dma_start(
                    out=langt2, in_=langt2_d.rearrange("p (c bm) -> p c bm", c=2)
                )
                nc.sync.dma_start(out=x01_1, in_=xtv2[:, :, 1])
                nc.gpsimd.dma_start(out=x2_1, in_=xtv[256:320, 1])

                # Dummy matmuls to flip the HAM clock gate (~3.4us of sustained
                # activity) while the first DMAs land.
                for wi in range(N_WARM):
                    pw = php0.tile([128, TILE], F32, tag="ph0", name=f"pw{wi}")
                    nc.tensor.matmul(
                        pw, warm[:, 0:128], warm, start=True, stop=True
                    )

                xbatches = {0: (x01_0, x2_0), 1: (x01_1, x2_1)}
                # batches 2-3 queue behind the fill-critical transfers on
                # sync (fresh pool slots, no WAR gate): the per-engine queue
                # FIFO starts them only after the fill-critical set
                for nbb in (2, 3):
                    x01n = xp.tile([128, 2, N], F16, tag="x01", bufs=3,
                                   name=f"x01_{nbb}")
                    x2n = xp.tile([64, N], F16, tag="x2", bufs=3,
                                  name=f"x2_{nbb}")
                    nc.sync.dma_start(out=x01n, in_=xtv2[:, :, nbb])
                    nc.sync.dma_start(out=x2n, in_=xtv[256:320, nbb])
                    xbatches[nbb] = (x01n, x2n)
                phs = {}

                def emit_l1(t):
                    # L1 stream-major: consecutive matmuls reuse the same
                    # moving tensor (x0 twice, x1 twice, x2 pair) -- a
                    # moving-stream switch costs ~85ns of PE issue time.
                    x01, x2 = xbatches[t // 2]
                    x0 = x01[:, 0, :]
                    x1 = x01[:, 1, :]
                    rs = slice((t % 2) * TILE, (t % 2 + 1) * TILE)
                    ph0 = php0.tile([128, TILE], F32, tag="ph0", name=f"ph0_{t}")
                    ph1 = php1.tile([128, TILE], F32, tag="ph1", name=f"ph1_{t}")
                    nc.tensor.matmul(
                        ph0, w3t[:, 0, 0:128], x0[:, rs], start=True, stop=False
                    )
                    nc.tensor.matmul(
                        ph1, w3t[:, 0, 128:256], x0[:, rs], start=True, stop=False
                    )
                    nc.tensor.matmul(
                        ph0, w3t[:, 1, 0:128], x1[:, rs], start=False, stop=False
                    )
                    nc.tensor.matmul(
                        ph1, w3t[:, 1, 128:256], x1[:, rs], start=False, stop=False
                    )
                    # the two K=32 tail matmuls sit in different PE
                    # row-groups (rows 0-31 / 32-63) and run concurrently
                    nc.tensor.matmul(
                        ph0, w3c2[0:32, :], x2[0:32, rs], start=False, stop=True
                    )
                    nc.tensor.matmul(
                        ph1, w3c2[32:64, :], x2[32:64, rs], start=False, stop=True
                    )
                    phs[t] = (ph0, ph1)

                # attention A's 16 small matmuls + exp run during the tail of
                # the x/weight DMAs, before tile 0's L1
                emit_attention_a()
                # bridge dummies keep the PE busy over the exp -> em -> attB
                # latency (an idle PE re-arms the HAM gate: ~10us half clock);
                # on the mv tag they also WAR-wait for the exp read of ps_sl,
                # which places them exactly in that hole
                for wi in range(N_BRIDGE):
                    pwb = pmv.tile([128, TILE], F32, tag="mv", name=f"pwb{wi}")
                    nc.tensor.matmul(
                        pwb, warm[:, 0:128], warm, start=True, stop=True
                    )
                # software pipeline: tile t+1's L1 is emitted BEFORE tile t's
                # h/L2, so the PE chews next tile's L1 while DVE/ACT turn
                # around h0/h1 -- the h0 latency leaves the critical cycle
                emit_l1(0)
                for t in range(NT):
                    bb = t // 2
                    if t % 2 == 1 and 4 <= (t + 3) // 2 < BPC:
                        # batch prefetch ahead of first use; with bufs=3 the
                        # WAR gate gives a ~3-tile lead
                        nbb = (t + 3) // 2
                        x01n = xp.tile([128, 2, N], F16, tag="x01", bufs=3,
                                       name=f"x01_{nbb}")
                        x2n = xp.tile([64, N], F16, tag="x2", bufs=3,
                                      name=f"x2_{nbb}")
                        nc.sync.dma_start(out=x01n, in_=xtv2[:, :, nbb])
                        nc.sync.dma_start(out=x2n, in_=xtv[256:320, nbb])
                        xbatches[nbb] = (x01n, x2n)
                    if t + 1 < NT:
                        emit_l1(t + 1)
                    if epi_pending:
                        # pair out-multiply at the DVE queue front: done
                        # (~0.7us in) before this iteration's mv needs the
                        # pair bank, and it fills the iteration whose h1
                        # rides ACT
                        emit_epilogue()
                    ph0, ph1 = phs.pop(t)
                    # h0 on DVE; h1 alternates DVE (even) / ACT (odd): with
                    # the deferred out-multiply this keeps every engine's
                    # per-iteration load ~2.1-2.4us, under the PE's ~2.7
                    h0 = wp.tile([128, TILE], F16, tag="h0")
                    nc.vector.tensor_scalar(
                        out=h0, in0=ph0,
                        scalar1=b3s[:, 0:1], scalar2=0.0,
                        op0=mybir.AluOpType.add, op1=mybir.AluOpType.max,
                    )
                    h1 = wp.tile([128, TILE], F16, tag="h1")
                    if t % 2 == 1 or t < 4:
                        nc.vector.tensor_scalar(
                            out=h1, in0=ph1,
                            scalar1=b3s[:, 1:2], scalar2=0.0,
                            op0=mybir.AluOpType.add, op1=mybir.AluOpType.max,
                        )
                    else:
                        nc.scalar.activation(
                            h1, ph1, AF.Relu, bias=b3s[:, 1:2]
                        )
                    if len(xq1_pending) > XQ_LAG - 1:
                        # both squares of tile t-XQ_LAG as ONE merged
                        # [128, 2*TILE] fp16 DVE op (2x mode): the xv
                        # input is guaranteed done, so the strict-FIFO
                        # DVE queue never stalls waiting on ACT.
                        nc.vector.tensor_mul(*xq1_pending.pop(0))
                    if t == 0:
                        # attention part B's 17 matmuls fill the PE gap
                        # while DVE computes h0/h1; af is then ready well
                        # before xv(0) needs it
                        emit_attention_b()
                    if len(pending) > 2 and pending[0][0] % 2 == 0:
                        # both tiles of a completed pair: 8 matvec matmuls
                        # as two 4-way concurrent column-group slots.  Both
                        # K-chunks stay in ONE iteration: an accumulation
                        # group left open across interleaved L1/L2 matmuls
                        # costs ~500ns/tile (measured).
                        emit_mv_c0(pending.pop(0), pending.pop(0))
                        emit_mv_c1()
                    # L2 ordered h0-first so it can start before h1 is done
                    po0 = pop0.tile([128, TILE], F32, tag="po0")
                    po1 = pop1.tile([128, TILE], F32, tag="po1")
                    nc.tensor.matmul(
                        po0, w4t[:, 0, 0:128], h0, start=True, stop=False
                    )
                    nc.tensor.matmul(
                        po1, w4t[:, 0, 128:256], h0, start=True, stop=False
                    )
                    nc.tensor.matmul(
                        po0, w4t[:, 1, 0:128], h1, start=False, stop=True
                    )
                    nc.tensor.matmul(
                        po1, w4t[:, 1, 128:256], h1, start=False, stop=True
                    )
                    xvt = wp.tile([128, 2, TILE], F16, tag="xv")
                    for o, po in ((0, po0), (1, po1)):
                        nc.scalar.activation(
                            xvt[:, o, :], po, AF.Relu,
                            bias=b4af[:, o, bb : bb + 1],
                            scale=af[:, o, bb : bb + 1],
                        )
                    xqt = wp.tile([128, 2, TILE], F16, tag="xq")
                    xq1_pending.append((xqt, xvt, xvt))
                    pending.append((t, [xvt[:, 0, :], xvt[:, 1, :]],
                                    [xqt[:, 0, :], xqt[:, 1, :]]))
                for q in xq1_pending:
                    nc.vector.tensor_mul(*q)
                xq1_pending.clear()
                while pending or mv_c1_pending:
                    if mv_c1_pending:
                        emit_mv_c1()
                    else:
                        emit_mv_c0(pending.pop(0), pending.pop(0))
                    while epi_pending:
                        emit_epilogue()
                nc.sync.dma_start(
                    out=outv[:, 0, NPAIR // 2 : NPAIR, :],
                    in_=out2[0:1, NPAIR // 2 : NPAIR, :],
                )
                nc.sync.dma_start(
                    out=outv[:, 1, NPAIR // 2 : NPAIR, :],
                    in_=out2[32:33, NPAIR // 2 : NPAIR, :],
                )
    nc.compile()
    return nc


_NC_CACHE = {}


def _get_nc():
    if "nc" not in _NC_CACHE:
        _NC_CACHE["nc"] = _build_nc()
    return _NC_CACHE["nc"]


def _f16(x):
    return np.ascontiguousarray(x).astype(np.float16)


def kernel(**inputs) -> np.ndarray:
    object_feat = np.ascontiguousarray(np.asarray(inputs["object_feat"], np.float32))
    lang_feat = np.ascontiguousarray(np.asarray(inputs["lang_feat"], np.float32))
    lang_mask = np.asarray(inputs["lang_mask"])
    W3 = np.asarray(inputs["W3"], np.float32)
    W4 = np.asarray(inputs["W4"], np.float32)
    b3 = np.asarray(inputs["b3"], np.float32)
    b4 = np.asarray(inputs["b4"], np.float32)
    Wa = np.asarray(inputs["Wa"], np.float32)
    Ws = np.asarray(inputs["Ws"], np.float32)
    bs = np.asarray(inputs["bs"], np.float32)

    w3r = _f16(W3[0:256].reshape(2, 128, D).transpose(1, 0, 2).reshape(128, 2 * D))
    w3c2 = np.concatenate([W3[256:288, 0:128], W3[256:288, 128:256]], axis=0)
    w3cr = _f16(w3c2)
    w4r = _f16(W4.reshape(2, 128, D).transpose(1, 0, 2).reshape(128, 2 * D))
    csth = np.zeros((128, 4), np.float16)
    csth[:, 0:2] = _f16(Ws[:, 0].reshape(2, 128).T)
    csth[:, 2:4] = _f16(Wa[D:, 0].reshape(2, 128).T)

    in_maps = []
    for i in range(NCORES):
        sl = slice(i * BPC, (i + 1) * BPC)
        of = object_feat[sl]                                   # (BPC, N, C1)
        lf = lang_feat[sl]                                     # (BPC, M, D)
        xt = of.reshape(R, C1).T
        xt_dup = np.concatenate([xt, xt[256:288]], axis=0)     # (320, R)
        cstf = np.zeros((128, 29), np.float32)
        cstf[:, 0:2] = b3.reshape(2, 128).T
        cstf[:, 2:4] = b4.reshape(2, 128).T
        cstf[0, 4] = bs[0]
        cstf[0:M, 5:13] = lang_mask[sl].T.astype(np.float32)
        # b4 chunk-major, replicated across the BPC batches: [128, 2, 8]
        cstf[:, 13:29] = np.repeat(
            b4.reshape(2, 128).T[:, :, None], BPC, axis=2
        ).reshape(128, 2 * BPC)
        lt2 = lf.transpose(2, 0, 1).reshape(2, 128, BPC * M)
        in_maps.append(
            {
                "xt": _f16(xt_dup),
                "langm": _f16(lf.transpose(1, 0, 2)),
                "langt2": _f16(lt2.transpose(1, 0, 2).reshape(128, 2 * BPC * M)),
                "cstf": cstf,
                "csth": csth,
                "w3": w3r,
                "w3c": w3cr,
                "w4": w4r,
            }
        )

    nc = _get_nc()
    res = run_bass_kernel_spmd(nc, in_maps, core_ids=list(range(NCORES)))
    _NC_CACHE["last_results"] = res
    out = np.empty((B, 1, N), np.float32)
    for i in range(NCORES):
        out[i * BPC : (i + 1) * BPC, 0, :] = res.results[i]["out"].reshape(BPC, N)
    out += bs[0]  # final bias applied on host (constant add)
    return out


# revision 85
# speedup vs baseline: 1.1043x; 1.1043x over previous
"""Trainium2 Bass kernel for nn_AttenModule (B=64, N=1024, M=80, C1=288, D=256).

Math notes (derived from the reference):
  score[b,n,m] = (oa@w_o)[b,n] + (lang@w_l)[b,m] + ba, softmax over m.
  The (oa@w_o)[b,n] and ba terms are constant along m, so they cancel in the
  softmax -> att[b,n,:] == softmax_m(mask(lang[b]@w_l)) is independent of n,
  and att_feat[b,:] = sum_m att[b,m]*lang[b,m,:] is a per-batch vector.
  Hence the entire W1/W2/w_o branch is dead.

  out = (v@Ws)/max(||v||,eps) + bs with v = relu(osc * af[b]) is
  scale-invariant in v (relu commutes with positive scales), so the softmax
  denominator only needs enough accuracy to keep fp16 ranges in check.

  Remaining per-row work (row = (b,n)):
    osc = relu(x @ W3 + b3) @ W4 + b4            # x = object_feat row (288,)
    v   = relu(osc * af[b])                      # af[b] = softmax(lang@w_l) @ lang
    out = (v @ Ws) / sqrt(||v||^2 + 1e-24) + bs

Device layout: feature-on-partition (transposed activations).  Per core
(8 cores, data-parallel over B): 8 batches = 8192 rows, row-tiles of 512.
All matmuls run in fp16 (11-bit mantissa, 1 cycle/row); PSUM accumulates
fp32.  Biases and the attention scale vector stay fp32 (applied via the
ACT engine's per-partition scale/bias).

Scheduling notes (v2):
  - Software pipelining: tile t+1's L1 matmuls are emitted before tile t's
    h/L2, so the PE runs next tile's L1 while DVE turns around h0/h1 and
    the h0 latency leaves the critical cycle (steady period ~2.7us/tile,
    PE-bound at ~11 matmul slots).
  - Engine split per tile: DVE h0/h1 (+merged xq at 1-tile lag, pair
    out-mul); ACT xv0/xv1 (+pair rsqrt); ACT keeps ~0.8us/tile of slack to
    drain its pipe-fill backlog (exp + early xv bursts).
  - Epilogues are batched per 2 tiles ("pair"): tile 2p's dot/ss go to PSUM
    partitions 0-31/64-95 of one bank (32-wide PE column groups), tile
    2p+1's to 32-63/96-127.  One ACT rsqrt over partitions 64..96 and one
    DVE multiply over 0..32 then serve both tiles (ACT/DVE cost is
    partition-count independent).  Matvecs trail the main loop by MV_LAG
    tiles so the ACT-produced xv/xq are always ready.
  - attention is split: scores+exp before the loop (PE/ACT), the
    mask/denominator/af chain inside tile 0 (where its PE matmuls fill the
    exp->em dependency bubble); psum tiles borrow the po0/po1 tags so the
    ph0/ph1 double-buffer rotation stays clean across tiles 0-2.
  - DMA: the 16 engines round-robin all non-empty HWDGE queues with no
    priorities, one queue sustains only ~110-180 GB/s, each dma_start costs
    ~700ns of serial issue time on its engine, and reused semaphores force
    issue-order waits.  So: the fill rides one deadline-ordered FIFO on
    sync (weights, batch-0 x in halves, langm, then batches 1-3), langt2
    alone on gpsimd, and batches 4-7 are emitted mid-loop where the xp-pool
    WAR gate self-paces them ~3 tiles ahead of use.
  - The HAM clock gate holds the PE at 1.2 GHz until ~3.4us of sustained
    activity and re-arms on any >~0.8us idle (costing ~10us of half clock):
    N_WARM dummy matmuls cover the first x/weight DMAs, and the mv-tagged
    bridge dummies WAR-wait on the exp read of ps_sl, which lands them
    exactly in the exp->em PE hole.
"""

import numpy as np

import concourse.bacc as bacc
import concourse.tile as tile
from concourse import mybir
from concourse.bass_utils import run_bass_kernel_spmd

B, N, M = 64, 1024, 80
C1, D = 288, 256
NCORES = 8
BPC = B // NCORES          # batches per core
R = BPC * N                # rows per core
TILE = 512
NT = R // TILE             # row tiles per core
NPAIR = NT // 2
F32 = mybir.dt.float32
F16 = mybir.dt.float16

N_WARM = 8                 # initial PE warm-up matmuls (HAM clock gate)
N_BRIDGE = 3               # bridge warmups over the attention exp->denom gap
XQ_LAG = 1                 # tiles of lag for the merged xq = xv*xv DVE op
MV_LAG = 3                 # tiles of lag for the matvec + epilogue


def _build_nc():
    nc = bacc.Bacc("TRN2", target_bir_lowering=False, debug=False)

    # xt rows 0..287 = x^T; rows 288..319 duplicate rows 256..287 so the two
    # K=32 tail matmuls (one per out-chunk) can run in concurrent PE row-groups
    xt_d = nc.dram_tensor("xt", [C1 + 32, R], F16, kind="ExternalInput").ap()
    langm_d = nc.dram_tensor("langm", [M, BPC, D], F16, kind="ExternalInput").ap()
    # langt2 pre-arranged as [128, 2, BPC*M] on the host
    langt2_d = nc.dram_tensor("langt2", [128, 2 * BPC * M], F16, kind="ExternalInput").ap()
    # w3/w4 pre-arranged as [128, 2, D] (chunk-major)
    w3_d = nc.dram_tensor("w3", [128, 2 * D], F16, kind="ExternalInput").ap()
    # w3c2: rows 0-31 = W3[256:288, 0:128], rows 32-63 = W3[256:288, 128:256]
    w3c_d = nc.dram_tensor("w3c", [64, 128], F16, kind="ExternalInput").ap()
    w4_d = nc.dram_tensor("w4", [128, 2 * D], F16, kind="ExternalInput").ap()
    # packed fp32 consts: cols [b3(2) | b4(2) | bs(1) | maskt(8, rows 0-79) |
    #                           b4rep(16 = b4 chunk-major replicated per batch)]
    cstf_d = nc.dram_tensor("cstf", [128, 29], F32, kind="ExternalInput").ap()
    # packed fp16 consts: cols [ws(2) | wl(2)]
    csth_d = nc.dram_tensor("csth", [128, 4], F16, kind="ExternalInput").ap()
    out_d = nc.dram_tensor("out", [1, R], F32, kind="ExternalOutput").ap()

    AF = mybir.ActivationFunctionType

    with tile.TileContext(nc) as tc:
        with tc.tile_pool(name="const", bufs=1) as cp:
            # csth/cstf at the HEAD of the sync queue: tiny, and csth feeds
            # the ws32 copies + attention scores
            csth = cp.tile([128, 4], F16)
            nc.sync.dma_start(out=csth, in_=csth_d)
            cstf = cp.tile([128, 29], F32)
            nc.sync.dma_start(out=cstf, in_=cstf_d)
            w3t = cp.tile([128, 2, D], F16)
            w3c2 = cp.tile([64, 128], F16)
            langt2 = cp.tile([128, 2, BPC, M], F16)
            w4t = cp.tile([128, 2, D], F16)
            langm = cp.tile([M, BPC, D], F16)
            wss = csth[:, 0:2]
            wls = csth[:, 2:4]
            b3s = cstf[:, 0:2]
            b4s = cstf[:, 2:4]
            maskt = cstf[0:M, 5:13]
            b4rep = cstf[:, 13:29].rearrange("p (c b) -> p c b", c=2)
            ones_m = cp.tile([M, 1], F16)
            nc.vector.memset(ones_m, 1.0)
            ones_1x128 = cp.tile([1, 128], F16)
            nc.vector.memset(ones_1x128, 1.0)
            ones_128 = cp.tile([128, 1], F16)
            nc.vector.memset(ones_128, 1.0)
            # 32-wide stationaries for the matvecs (col 0 live, rest zero):
            # each dot/ss matmul then fills a whole 32-partition PE column
            # group, so every psum partition the batched epilogue reads is
            # written (same PE cost -- it scales with the moving free size)
            ws32 = cp.tile([128, 2, 32], F16)
            nc.vector.memset(ws32, 0.0)
            nc.vector.tensor_copy(ws32[:, 0, 0:1], wss[:, 0:1])
            nc.vector.tensor_copy(ws32[:, 1, 0:1], wss[:, 1:2])
            ones32 = cp.tile([128, 32], F16)
            nc.vector.memset(ones32, 0.0)
            nc.vector.memset(ones32[:, 0:1], 1.0)
            eps_sb = cp.tile([33, 1], F32)
            nc.vector.memset(eps_sb, 1e-24)
            # PE warm-up: the HAM clock gate keeps the PE at 1.2 GHz until it
            # sees ~3.4us of sustained activity, and re-throttles (costing
            # ~10us of half-speed) if the PE goes idle again.
            warm = cp.tile([128, TILE], F16)
            nc.gpsimd.memset(warm, 0.0)

            # ---------- main loop (attention interleaved into it) ----------
            with (
                # x tiles use bufs=3 (per-tag): the WAR gate then self-paces
                # batch bb's DMA ~3 tiles ahead of its first use
                tc.tile_pool(name="xt", bufs=3) as xp,
                tc.tile_pool(name="work", bufs=4) as wp,
                tc.tile_pool(name="ep", bufs=3) as epp,
                tc.tile_pool(name="ph0p", bufs=2, space="PSUM") as php0,
                tc.tile_pool(name="ph1p", bufs=2, space="PSUM") as php1,
                tc.tile_pool(name="po0p", bufs=2, space="PSUM") as pop0,
                tc.tile_pool(name="po1p", bufs=1, space="PSUM") as pop1,
                tc.tile_pool(name="pmv", bufs=1, space="PSUM") as pmv,
            ):
                # out2[0, p, :] = tile 2p's outputs, out2[32, p, :] = tile 2p+1's
                out2 = cp.tile([33, NPAIR, TILE], F32)
                # DRAM view [1, 2, NPAIR, TILE]: index [o, k, p, r] = tile 2p+k
                outv = out_d.rearrange("o (p k r) -> o k p r", k=2, r=TILE)
                xtv = xt_d.rearrange("c (bb r) -> c bb r", bb=BPC)

                e_sb = cp.tile([M, BPC], F32)
                em_sb = cp.tile([M, BPC], F16)
                rd32 = cp.tile([1, BPC], F32)
                rdf = cp.tile([1, BPC], F16)
                af = cp.tile([128, 2, BPC], F32)
                b4af = cp.tile([128, 2, BPC], F32)

                def emit_attention_a():
                    # --- attention part A: scores, exp, mask.  The denom
                    # matmul lives in part B: here it would block the PE
                    # queue on the exp->em latency right when tile 0's L1
                    # could run.
                    ps_sl = pmv.tile([M, BPC], F32, tag="mv")
                    for b in range(BPC):
                        for c in range(2):
                            nc.tensor.matmul(
                                ps_sl[:, b : b + 1],
                                langt2[:, c, b, :],
                                wls[:, c : c + 1],
                                start=(c == 0),
                                stop=(c == 1),
                            )
                    nc.scalar.activation(e_sb, ps_sl, AF.Exp)

                def emit_attention_b():
                    # em (exp * mask) lives here, not in part A: on the
                    # in-order DVE queue it would otherwise block tile 0's
                    # h0 behind the exp latency
                    nc.vector.tensor_mul(em_sb, e_sb, maskt)
                    # denom per batch + reciprocal (the per-batch scale cancels
                    # in the output; it only keeps fp16 magnitudes in range)
                    ps_dn = pmv.tile([1, BPC], F32, tag="mv")
                    nc.tensor.matmul(ps_dn, ones_m, em_sb, start=True, stop=True)
                    nc.vector.reciprocal(rd32, ps_dn)
                    nc.vector.tensor_copy(rdf, rd32)
                    # --- attention part B: af matmuls + scaling.  Emitted
                    # after tile 0's L1 so its PE/DVE work fills the
                    # exp->em dependency bubble instead of stalling the head.
                    # psum tiles borrow the po0/po1 tags: they are dead by the
                    # time tile 0's L2 needs the banks, and ph0/ph1 rotation
                    # stays clean for tiles 1..2.
                    ps_rdb = pop0.tile([128, BPC], F32, tag="po0")
                    nc.tensor.matmul(ps_rdb, ones_1x128, rdf, start=True, stop=True)
                    rdb = cp.tile([128, BPC], F32)
                    nc.vector.tensor_copy(rdb, ps_rdb)
                    ps_af = pop1.tile([128, 2, BPC], F32, tag="po1")
                    for b in range(BPC):
                        for c in range(2):
                            nc.tensor.matmul(
                                ps_af[:, c, b : b + 1],
                                langm[:, b, c * 128 : (c + 1) * 128],
                                em_sb[:, b : b + 1],
                                start=True,
                                stop=True,
                            )
                    for c in range(2):
                        nc.vector.tensor_mul(af[:, c, :], ps_af[:, c, :], rdb)
                    nc.vector.tensor_mul(b4af, af, b4rep)

                mv_c1_pending = []

                def _mv_group(pr, c, xsa, xqa, xsb, xqb):
                    # one K-chunk of BOTH tiles' matvecs as a 4-way
                    # concurrent PE column-group issue (measured: 4 groups
                    # with different moving streams issue within ~16ns and
                    # overlap fully -> 1 slot instead of 2):
                    #   tile 2p:   dot -> partitions 0-31,  ss -> 64-95
                    #   tile 2p+1: dot -> partitions 32-63, ss -> 96-127
                    # skip_group_check: the sim's zero-region tracker maps
                    # partition offsets to overlapping phantom rows for
                    # 32-wide column-group outputs; the real groups are
                    # partition-disjoint.
                    st = {"start": c == 0, "stop": c == 1,
                          "skip_group_check": True}
                    nc.tensor.matmul(
                        pr[0:32, :], ws32[:, c, :], xsa[c],
                        tile_position=(0, 0), **st,
                    )
                    nc.tensor.matmul(
                        pr[32:64, :], ws32[:, c, :], xsb[c],
                        tile_position=(0, 32), **st,
                    )
                    nc.tensor.matmul(
                        pr[64:96, :], ones32, xqa[c],
                        tile_position=(0, 64), **st,
                    )
                    nc.tensor.matmul(
                        pr[96:128, :], ones32, xqb[c],
                        tile_position=(0, 96), **st,
                    )

                def emit_mv_c0(a, b):
                    # first K-chunk in this iteration; second chunk + rsqrt
                    # deferred to the next -> every iteration carries exactly
                    # one 4-way mv slot (10 PE slots flat)
                    ta, xsa, xqa = a
                    tb, xsb, xqb = b
                    p = ta // 2
                    pr = pmv.tile([128, TILE], F32, tag="mv", name=f"pair{p}")
                    _mv_group(pr, 0, xsa, xqa, xsb, xqb)
                    mv_c1_pending.append((p, pr, xsa, xqa, xsb, xqb))

                def emit_mv_c1():
                    p, pr, xsa, xqa, xsb, xqb = mv_c1_pending.pop(0)
                    _mv_group(pr, 1, xsa, xqa, xsb, xqb)
                    if True:
                        # batched epilogue for the pair: one rsqrt over
                        # partitions 64..96 (both ss rows; ACT cost is
                        # partition-count independent), one multiply over
                        # partitions 0..32 (both dot rows).  Unwritten
                        # partitions in between produce garbage lanes that
                        # are never stored.  The multiply is deferred to the
                        # FRONT of the next iteration's DVE queue: it balances
                        # the per-iteration DVE load (which otherwise exceeds
                        # the PE period on epilogue iterations) while still
                        # completing before the pair bank's WAR reuse.
                        rc = epp.tile([33, TILE], F32, tag="rc")
                        inst = nc.scalar.activation(
                            rc, pr[64:97, :], AF.Sqrt, bias=eps_sb
                        )
                        inst.ins.func = AF.Rsqrt
                        epi_pending.append((p, pr, rc))

                pending = []
                xq1_pending = []
                epi_pending = []

                def emit_epilogue():
                    p, pr, rc = epi_pending.pop(0)
                    nc.vector.tensor_mul(out2[:, p, :], pr[0:33, :], rc)
                    if p == NPAIR // 2 - 1:
                        # first half of the output: overlap store
                        nc.sync.dma_start(
                            out=outv[:, 0, 0 : NPAIR // 2, :],
                            in_=out2[0:1, 0 : NPAIR // 2, :],
                        )
                        nc.sync.dma_start(
                            out=outv[:, 1, 0 : NPAIR // 2, :],
                            in_=out2[32:33, 0 : NPAIR // 2, :],
                        )
                xtv2 = xt_d[0:256, :].rearrange(
                    "(c p) (bb r) -> p c bb r", c=2, bb=BPC
                )

                # --- batch-0/1 loads.  Queue priorities (Q0=gpsimd,
                # Q10=scalar, Q1=sync; the 16 DMA engines round-robin across
                # queues, so the critical fill tensors must not sit behind
                # bulk x):
                #   gpsimd: langt2 FIRST (gates attention A), x first halves
                #           (tile 0's L1), then x second halves (tile 1)
                #   scalar: consts; batch-1 x is issued AFTER attention A's
                #           exp in the in-order scalar stream, so it starts
                #           flowing only once the critical fill has landed
                #   sync:   w3t (gates L1), langm (attention B), w4t (L2),
                #           then the bb>=2 batch loads (throttled by xp bufs)
                x01_0 = xp.tile([128, 2, N], F16, tag="x01", name="x01_0", bufs=3)
                x2_0 = xp.tile([64, N], F16, tag="x2", name="x2_0", bufs=3)
                x01_1 = xp.tile([128, 2, N], F16, tag="x01", name="x01_1", bufs=3)
                x2_1 = xp.tile([64, N], F16, tag="x2", name="x2_1", bufs=3)
                # Fill transfers: one deadline-ordered FIFO on sync, with
                # langt2 alone on the gpsimd queue in parallel.  (Spreading
                # descriptors over 3 queues accelerates tiles 0-1 but starves
                # the batch stream for tiles 2-6: net +8us.  The sync engine's
                # ~700ns-per-descriptor issue rate does leave 103-181 GB/s
                # dips between small descriptors, but every rebalance tried
                # costs more mid-ramp than it saves up front.)
                nc.sync.dma_start(out=w3t, in_=w3_d.rearrange("p (c d) -> p c d", c=2))
                nc.gpsimd.dma_start(
                    out=langt2, in_=langt2_d.rearrange("p (c bm) -> p c bm", c=2)
                )
                nc.sync.dma_start(out=x01_0[:, 0, 0:TILE], in_=xtv2[:, 0, 0, 0:TILE])
                nc.sync.dma_start(out=x01_0[:, 1, 0:TILE], in_=xtv2[:, 1, 0, 0:TILE])
                nc.sync.dma_start(out=x2_0[:, 0:TILE], in_=xtv[256:320, 0, 0:TILE])
                nc.sync.dma_start(out=w3c2, in_=w3c_d)
                nc.sync.dma_start(out=x01_0[:, 0, TILE:N], in_=xtv2[:, 0, 0, TILE:N])
                nc.sync.dma_start(out=x01_0[:, 1, TILE:N], in_=xtv2[:, 1, 0, TILE:N])
                nc.sync.dma_start(out=x2_0[:, TILE:N], in_=xtv[256:320, 0, TILE:N])
                nc.sync.dma_start(out=langm, in_=langm_d)
                nc.sync.dma_start(out=w4t, in_=w4_d.rearrange("p (c d) -> p c d", c=2))
                nc.sync.dma_start(out=x01_1, in_=xtv2[:, :, 1])
                nc.sync.dma_start(out=x2_1, in_=xtv[256:320, 1])

                # Dummy matmuls to flip the HAM clock gate (~3.4us of sustained
                # activity) while the first DMAs land.
                for wi in range(N_WARM):
                    pw = php0.tile([128, TILE], F32, tag="ph0", name=f"pw{wi}")
                    nc.tensor.matmul(
                        pw, warm[:, 0:128], warm, start=True, stop=True
                    )

                xbatches = {0: (x01_0, x2_0), 1: (x01_1, x2_1)}
                # batches 2-3 queue behind the fill-critical transfers on
                # sync (fresh pool slots, no WAR gate): the per-engine queue
                # FIFO starts them only after the fill-critical set
                for nbb in (2, 3):
                    x01n = xp.tile([128, 2, N], F16, tag="x01", bufs=3,
                                   name=f"x01_{nbb}")
                    x2n = xp.tile([64, N], F16, tag="x2", bufs=3,
                                  name=f"x2_{nbb}")
                    nc.sync.dma_start(out=x01n, in_=xtv2[:, :, nbb])
                    nc.sync.dma_start(out=x2n, in_=xtv[256:320, nbb])
                    xbatches[nbb] = (x01n, x2n)
                phs = {}

                def emit_l1(t):
                    # L1 stream-major: consecutive matmuls reuse the same
                    # moving tensor (x0 twice, x1 twice, x2 pair) -- a
                    # moving-stream switch costs ~85ns of PE issue time.
                    x01, x2 = xbatches[t // 2]
                    x0 = x01[:, 0, :]
                    x1 = x01[:, 1, :]
                    rs = slice((t % 2) * TILE, (t % 2 + 1) * TILE)
                    ph0 = php0.tile([128, TILE], F32, tag="ph0", name=f"ph0_{t}")
                    ph1 = php1.tile([128, TILE], F32, tag="ph1", name=f"ph1_{t}")
                    nc.tensor.matmul(
                        ph0, w3t[:, 0, 0:128], x0[:, rs], start=True, stop=False
                    )
                    nc.tensor.matmul(
                        ph1, w3t[:, 0, 128:256], x0[:, rs], start=True, stop=False
                    )
                    nc.tensor.matmul(
                        ph0, w3t[:, 1, 0:128], x1[:, rs], start=False, stop=False
                    )
                    nc.tensor.matmul(
                        ph1, w3t[:, 1, 128:256], x1[:, rs], start=False, stop=False
                    )
                    # the two K=32 tail matmuls sit in different PE
                    # row-groups (rows 0-31 / 32-63) and run concurrently
                    nc.tensor.matmul(
                        ph0, w3c2[0:32, :], x2[0:32, rs], start=False, stop=True
                    )
                    nc.tensor.matmul(
                        ph1, w3c2[32:64, :], x2[32:64, rs], start=False, stop=True
                    )
                    phs[t] = (ph0, ph1)

                # attention A's 16 small matmuls + exp run during the tail of
                # the x/weight DMAs, before tile 0's L1
                emit_attention_a()
                # bridge dummies keep the PE busy over the exp -> em -> attB
                # latency (an idle PE re-arms the HAM gate: ~10us half clock);
                # on the mv tag they also WAR-wait for the exp read of ps_sl,
                # which places them exactly in that hole
                for wi in range(N_BRIDGE):
                    pwb = pmv.tile([128, TILE], F32, tag="mv", name=f"pwb{wi}")
                    nc.tensor.matmul(
                        pwb, warm[:, 0:128], warm, start=True, stop=True
                    )
                # software pipeline: tile t+1's L1 is emitted BEFORE tile t's
                # h/L2, so the PE chews next tile's L1 while DVE/ACT turn
                # around h0/h1 -- the h0 latency leaves the critical cycle
                emit_l1(0)
                for t in range(NT):
                    bb = t // 2
                    if t % 2 == 1 and 4 <= (t + 3) // 2 < BPC:
                        # batch prefetch ahead of first use; with bufs=3 the
                        # WAR gate gives a ~3-tile lead
                        nbb = (t + 3) // 2
                        x01n = xp.tile([128, 2, N], F16, tag="x01", bufs=3,
                                       name=f"x01_{nbb}")
                        x2n = xp.tile([64, N], F16, tag="x2", bufs=3,
                                      name=f"x2_{nbb}")
                        nc.sync.dma_start(out=x01n, in_=xtv2[:, :, nbb])
                        nc.sync.dma_start(out=x2n, in_=xtv[256:320, nbb])
                        xbatches[nbb] = (x01n, x2n)
                    if t + 1 < NT:
                        emit_l1(t + 1)
                    if epi_pending:
                        # pair out-multiply at the DVE queue front: done
                        # (~0.7us in) before this iteration's mv needs the
                        # pair bank, and it fills the iteration whose h1
                        # rides ACT
                        emit_epilogue()
                    ph0, ph1 = phs.pop(t)
                    # h0 on DVE; h1 alternates DVE (even) / ACT (odd): with
                    # the deferred out-multiply this keeps every engine's
                    # per-iteration load ~2.1-2.4us, under the PE's ~2.7
                    h0 = wp.tile([128, TILE], F16, tag="h0")
                    nc.vector.tensor_scalar(
                        out=h0, in0=ph0,
                        scalar1=b3s[:, 0:1], scalar2=0.0,
                        op0=mybir.AluOpType.add, op1=mybir.AluOpType.max,
                    )
                    h1 = wp.tile([128, TILE], F16, tag="h1")
                    if t % 2 == 1 or t < 4:
                        nc.vector.tensor_scalar(
                            out=h1, in0=ph1,
                            scalar1=b3s[:, 1:2], scalar2=0.0,
                            op0=mybir.AluOpType.add, op1=mybir.AluOpType.max,
                        )
                    else:
                        nc.scalar.activation(
                            h1, ph1, AF.Relu, bias=b3s[:, 1:2]
                        )
                    if len(xq1_pending) > XQ_LAG - 1:
                        # both squares of tile t-XQ_LAG as ONE merged
                        # [128, 2*TILE] fp16 DVE op (2x mode): the xv
                        # input is guaranteed done, so the strict-FIFO
                        # DVE queue never stalls waiting on ACT.
                        nc.vector.tensor_mul(*xq1_pending.pop(0))
                    if t == 0:
                        # attention part B's 17 matmuls fill the PE gap
                        # while DVE computes h0/h1; af is then ready well
                        # before xv(0) needs it
                        emit_attention_b()
                    if len(pending) > 2 and pending[0][0] % 2 == 0:
                        # both tiles of a completed pair: 8 matvec matmuls
                        # as two 4-way concurrent column-group slots.  Both
                        # K-chunks stay in ONE iteration: an accumulation
                        # group left open across interleaved L1/L2 matmuls
                        # costs ~500ns/tile (measured).
                        emit_mv_c0(pending.pop(0), pending.pop(0))
                        emit_mv_c1()
                    # L2 ordered h0-first so it can start before h1 is done
                    po0 = pop0.tile([128, TILE], F32, tag="po0")
                    po1 = pop1.tile([128, TILE], F32, tag="po1")
                    nc.tensor.matmul(
                        po0, w4t[:, 0, 0:128], h0, start=True, stop=False
                    )
                    nc.tensor.matmul(
                        po1, w4t[:, 0, 128:256], h0, start=True, stop=False
                    )
                    nc.tensor.matmul(
                        po0, w4t[:, 1, 0:128], h1, start=False, stop=True
                    )
                    nc.tensor.matmul(
                        po1, w4t[:, 1, 128:256], h1, start=False, stop=True
                    )
                    xvt = wp.tile([128, 2, TILE], F16, tag="xv")
                    for o, po in ((0, po0), (1, po1)):
                        nc.scalar.activation(
                            xvt[:, o, :], po, AF.Relu,
                            bias=b4af[:, o, bb : bb + 1],
                            scale=af[:, o, bb : bb + 1],
                        )
                    xqt = wp.tile([128, 2, TILE], F16, tag="xq")
                    xq1_pending.append((xqt, xvt, xvt))
                    pending.append((t, [xvt[:, 0, :], xvt[:, 1, :]],
                                    [xqt[:, 0, :], xqt[:, 1, :]]))
                for q in xq1_pending:
                    nc.vector.tensor_mul(*q)
                xq1_pending.clear()
                while pending or mv_c1_pending:
                    if mv_c1_pending:
                        emit_mv_c1()
                    else:
                        emit_mv_c0(pending.pop(0), pending.pop(0))
                    while epi_pending:
                        emit_epilogue()
                nc.sync.dma_start(
                    out=outv[:, 0, NPAIR // 2 : NPAIR, :],
                    in_=out2[0:1, NPAIR // 2 : NPAIR, :],
                )
                nc.sync.dma_start(
                    out=outv[:, 1, NPAIR // 2 : NPAIR, :],
                    in_=out2[32:33, NPAIR // 2 : NPAIR, :],
                )
    nc.compile()
    return nc


_NC_CACHE = {}


def _get_nc():
    if "nc" not in _NC_CACHE:
        _NC_CACHE["nc"] = _build_nc()
    return _NC_CACHE["nc"]


def _f16(x):
    return np.ascontiguousarray(x).astype(np.float16)


def kernel(**inputs) -> np.ndarray:
    object_feat = np.ascontiguousarray(np.asarray(inputs["object_feat"], np.float32))
    lang_feat = np.ascontiguousarray(np.asarray(inputs["lang_feat"], np.float32))
    lang_mask = np.asarray(inputs["lang_mask"])
    W3 = np.asarray(inputs["W3"], np.float32)
    W4 = np.asarray(inputs["W4"], np.float32)
    b3 = np.asarray(inputs["b3"], np.float32)
    b4 = np.asarray(inputs["b4"], np.float32)
    Wa = np.asarray(inputs["Wa"], np.float32)
    Ws = np.asarray(inputs["Ws"], np.float32)
    bs = np.asarray(inputs["bs"], np.float32)

    w3r = _f16(W3[0:256].reshape(2, 128, D).transpose(1, 0, 2).reshape(128, 2 * D))
    w3c2 = np.concatenate([W3[256:288, 0:128], W3[256:288, 128:256]], axis=0)
    w3cr = _f16(w3c2)
    w4r = _f16(W4.reshape(2, 128, D).transpose(1, 0, 2).reshape(128, 2 * D))
    csth = np.zeros((128, 4), np.float16)
    csth[:, 0:2] = _f16(Ws[:, 0].reshape(2, 128).T)
    csth[:, 2:4] = _f16(Wa[D:, 0].reshape(2, 128).T)

    in_maps = []
    for i in range(NCORES):
        sl = slice(i * BPC, (i + 1) * BPC)
        of = object_feat[sl]                                   # (BPC, N, C1)
        lf = lang_feat[sl]                                     # (BPC, M, D)
        xt = of.reshape(R, C1).T
        xt_dup = np.concatenate([xt, xt[256:288]], axis=0)     # (320, R)
        cstf = np.zeros((128, 29), np.float32)
        cstf[:, 0:2] = b3.reshape(2, 128).T
        cstf[:, 2:4] = b4.reshape(2, 128).T
        cstf[0, 4] = bs[0]
        cstf[0:M, 5:13] = lang_mask[sl].T.astype(np.float32)
        # b4 chunk-major, replicated across the BPC batches: [128, 2, 8]
        cstf[:, 13:29] = np.repeat(
            b4.reshape(2, 128).T[:, :, None], BPC, axis=2
        ).reshape(128, 2 * BPC)
        lt2 = lf.transpose(2, 0, 1).reshape(2, 128, BPC * M)
        in_maps.append(
            {
                "xt": _f16(xt_dup),
                "langm": _f16(lf.transpose(1, 0, 2)),
                "langt2": _f16(lt2.transpose(1, 0, 2).reshape(128, 2 * BPC * M)),
                "cstf": cstf,
                "csth": csth,
                "w3": w3r,
                "w3c": w3cr,
                "w4": w4r,
            }
        )

    nc = _get_nc()
    res = run_bass_kernel_spmd(nc, in_maps, core_ids=list(range(NCORES)))
    _NC_CACHE["last_results"] = res
    out = np.empty((B, 1, N), np.float32)
    for i in range(NCORES):
        out[i * BPC : (i + 1) * BPC, 0, :] = res.results[i]["out"].reshape(BPC, N)
    out += bs[0]  # final bias applied on host (constant add)
    return out


# revision 86
# speedup vs baseline: 1.1462x; 1.0379x over previous
"""Trainium2 Bass kernel for nn_AttenModule (B=64, N=1024, M=80, C1=288, D=256).

Math notes (derived from the reference):
  score[b,n,m] = (oa@w_o)[b,n] + (lang@w_l)[b,m] + ba, softmax over m.
  The (oa@w_o)[b,n] and ba terms are constant along m, so they cancel in the
  softmax -> att[b,n,:] == softmax_m(mask(lang[b]@w_l)) is independent of n,
  and att_feat[b,:] = sum_m att[b,m]*lang[b,m,:] is a per-batch vector.
  Hence the entire W1/W2/w_o branch is dead.

  out = (v@Ws)/max(||v||,eps) + bs with v = relu(osc * af[b]) is
  scale-invariant in v (relu commutes with positive scales), so the softmax
  denominator only needs enough accuracy to keep fp16 ranges in check.

  Remaining per-row work (row = (b,n)):
    osc = relu(x @ W3 + b3) @ W4 + b4            # x = object_feat row (288,)
    v   = relu(osc * af[b])                      # af[b] = softmax(lang@w_l) @ lang
    out = (v @ Ws) / sqrt(||v||^2 + 1e-24) + bs

Device layout: feature-on-partition (transposed activations).  Per core
(8 cores, data-parallel over B): 8 batches = 8192 rows, row-tiles of 512.
All matmuls run in fp16 (11-bit mantissa, 1 cycle/row); PSUM accumulates
fp32.  Biases and the attention scale vector stay fp32 (applied via the
ACT engine's per-partition scale/bias).

Scheduling notes (v2):
  - Software pipelining: tile t+1's L1 matmuls are emitted before tile t's
    h/L2, so the PE runs next tile's L1 while DVE turns around h0/h1 and
    the h0 latency leaves the critical cycle (steady period ~2.7us/tile,
    PE-bound at ~11 matmul slots).
  - Engine split per tile: DVE h0/h1 (+merged xq at 1-tile lag, pair
    out-mul); ACT xv0/xv1 (+pair rsqrt); ACT keeps ~0.8us/tile of slack to
    drain its pipe-fill backlog (exp + early xv bursts).
  - Epilogues are batched per 2 tiles ("pair"): tile 2p's dot/ss go to PSUM
    partitions 0-31/64-95 of one bank (32-wide PE column groups), tile
    2p+1's to 32-63/96-127.  One ACT rsqrt over partitions 64..96 and one
    DVE multiply over 0..32 then serve both tiles (ACT/DVE cost is
    partition-count independent).  Matvecs trail the main loop by MV_LAG
    tiles so the ACT-produced xv/xq are always ready.
  - attention is split: scores+exp before the loop (PE/ACT), the
    mask/denominator/af chain inside tile 0 (where its PE matmuls fill the
    exp->em dependency bubble); psum tiles borrow the po0/po1 tags so the
    ph0/ph1 double-buffer rotation stays clean across tiles 0-2.
  - DMA: the 16 engines round-robin all non-empty HWDGE queues with no
    priorities, one queue sustains only ~110-180 GB/s, each dma_start costs
    ~700ns of serial issue time on its engine, and reused semaphores force
    issue-order waits.  So: the fill rides one deadline-ordered FIFO on
    sync (weights, batch-0 x in halves, langm, then batches 1-3), langt2
    alone on gpsimd, and batches 4-7 are emitted mid-loop where the xp-pool
    WAR gate self-paces them ~3 tiles ahead of use.
  - The HAM clock gate holds the PE at 1.2 GHz until ~3.4us of sustained
    activity and re-arms on any >~0.8us idle (costing ~10us of half clock):
    N_WARM dummy matmuls cover the first x/weight DMAs, and the mv-tagged
    bridge dummies WAR-wait on the exp read of ps_sl, which lands them
    exactly in the exp->em PE hole.
"""

import numpy as np

import concourse.bacc as bacc
import concourse.tile as tile
from concourse import mybir
from concourse.bass_utils import run_bass_kernel_spmd

B, N, M = 64, 1024, 80
C1, D = 288, 256
NCORES = 8
BPC = B // NCORES          # batches per core
R = BPC * N                # rows per core
TILE = 512
NT = R // TILE             # row tiles per core
NPAIR = NT // 2
F32 = mybir.dt.float32
F16 = mybir.dt.float16

N_WARM = 8                 # initial PE warm-up matmuls (HAM clock gate)
N_BRIDGE = 0               # bridge warmups over the attention exp->denom gap
XQ_LAG = 1                 # tiles of lag for the merged xq = xv*xv DVE op
MV_LAG = 3                 # tiles of lag for the matvec + epilogue


def _build_nc():
    nc = bacc.Bacc("TRN2", target_bir_lowering=False, debug=False)

    # xt rows 0..287 = x^T; rows 288..319 duplicate rows 256..287 so the two
    # K=32 tail matmuls (one per out-chunk) can run in concurrent PE row-groups
    xt_d = nc.dram_tensor("xt", [C1 + 32, R], F16, kind="ExternalInput").ap()
    langm_d = nc.dram_tensor("langm", [M, BPC, D], F16, kind="ExternalInput").ap()
    # langt2 pre-arranged as [128, 2, BPC*M] on the host
    langt2_d = nc.dram_tensor("langt2", [128, 2 * BPC * M], F16, kind="ExternalInput").ap()
    # w3/w4 pre-arranged as [128, 2, D] (chunk-major)
    w3_d = nc.dram_tensor("w3", [128, 2 * D], F16, kind="ExternalInput").ap()
    # w3c2: rows 0-31 = W3[256:288, 0:128], rows 32-63 = W3[256:288, 128:256]
    w3c_d = nc.dram_tensor("w3c", [64, 128], F16, kind="ExternalInput").ap()
    w4_d = nc.dram_tensor("w4", [128, 2 * D], F16, kind="ExternalInput").ap()
    # packed fp32 consts: cols [b3(2) | b4(2) | bs(1) | maskt(8, rows 0-79) |
    #                           b4rep(16 = b4 chunk-major replicated per batch)]
    cstf_d = nc.dram_tensor("cstf", [128, 29], F32, kind="ExternalInput").ap()
    # packed fp16 consts: cols [ws(2) | wl(2)]
    csth_d = nc.dram_tensor("csth", [128, 4], F16, kind="ExternalInput").ap()
    out_d = nc.dram_tensor("out", [1, R], F32, kind="ExternalOutput").ap()

    AF = mybir.ActivationFunctionType

    with tile.TileContext(nc) as tc:
        with tc.tile_pool(name="const", bufs=1) as cp:
            # csth/cstf at the HEAD of the sync queue: tiny, and csth feeds
            # the ws32 copies + attention scores
            csth = cp.tile([128, 4], F16)
            nc.sync.dma_start(out=csth, in_=csth_d)
            cstf = cp.tile([128, 29], F32)
            nc.sync.dma_start(out=cstf, in_=cstf_d)
            w3t = cp.tile([128, 2, D], F16)
            w3c2 = cp.tile([64, 128], F16)
            langt2 = cp.tile([128, 2, BPC, M], F16)
            w4t = cp.tile([128, 2, D], F16)
            langm = cp.tile([M, BPC, D], F16)
            wss = csth[:, 0:2]
            wls = csth[:, 2:4]
            b3s = cstf[:, 0:2]
            b4s = cstf[:, 2:4]
            maskt = cstf[0:M, 5:13]
            b4rep = cstf[:, 13:29].rearrange("p (c b) -> p c b", c=2)
            ones_m = cp.tile([M, 1], F16)
            nc.vector.memset(ones_m, 1.0)
            ones_1x128 = cp.tile([1, 128], F16)
            nc.vector.memset(ones_1x128, 1.0)
            ones_128 = cp.tile([128, 1], F16)
            nc.vector.memset(ones_128, 1.0)
            # 32-wide stationaries for the matvecs (col 0 live, rest zero):
            # each dot/ss matmul then fills a whole 32-partition PE column
            # group, so every psum partition the batched epilogue reads is
            # written (same PE cost -- it scales with the moving free size)
            ws32 = cp.tile([128, 2, 32], F16)
            nc.vector.memset(ws32, 0.0)
            nc.vector.tensor_copy(ws32[:, 0, 0:1], wss[:, 0:1])
            nc.vector.tensor_copy(ws32[:, 1, 0:1], wss[:, 1:2])
            ones32 = cp.tile([128, 32], F16)
            nc.vector.memset(ones32, 0.0)
            nc.vector.memset(ones32[:, 0:1], 1.0)
            eps_sb = cp.tile([33, 1], F32)
            nc.vector.memset(eps_sb, 1e-24)
            # PE warm-up: the HAM clock gate keeps the PE at 1.2 GHz until it
            # sees ~3.4us of sustained activity, and re-throttles (costing
            # ~10us of half-speed) if the PE goes idle again.
            warm = cp.tile([128, TILE], F16)
            nc.gpsimd.memset(warm, 0.0)

            # ---------- main loop (attention interleaved into it) ----------
            with (
                # x tiles use bufs=3 (per-tag): the WAR gate then self-paces
                # batch bb's DMA ~3 tiles ahead of its first use
                tc.tile_pool(name="xt", bufs=3) as xp,
                tc.tile_pool(name="work", bufs=4) as wp,
                tc.tile_pool(name="ep", bufs=3) as epp,
                tc.tile_pool(name="ph0p", bufs=2, space="PSUM") as php0,
                tc.tile_pool(name="ph1p", bufs=2, space="PSUM") as php1,
                tc.tile_pool(name="po0p", bufs=2, space="PSUM") as pop0,
                tc.tile_pool(name="po1p", bufs=1, space="PSUM") as pop1,
                tc.tile_pool(name="pmv", bufs=1, space="PSUM") as pmv,
            ):
                # out2[0, p, :] = tile 2p's outputs, out2[32, p, :] = tile 2p+1's
                out2 = cp.tile([33, NPAIR, TILE], F32)
                # DRAM view [1, 2, NPAIR, TILE]: index [o, k, p, r] = tile 2p+k
                outv = out_d.rearrange("o (p k r) -> o k p r", k=2, r=TILE)
                xtv = xt_d.rearrange("c (bb r) -> c bb r", bb=BPC)

                e_sb = cp.tile([M, BPC], F32)
                em_sb = cp.tile([M, BPC], F16)
                rd32 = cp.tile([1, BPC], F32)
                rdf = cp.tile([1, BPC], F16)
                af = cp.tile([128, 2, BPC], F32)
                b4af = cp.tile([128, 2, BPC], F32)

                def emit_attention_a():
                    # --- attention part A: scores, exp, mask.  The denom
                    # matmul lives in part B: here it would block the PE
                    # queue on the exp->em latency right when tile 0's L1
                    # could run.
                    ps_sl = pmv.tile([M, BPC], F32, tag="mv")
                    for b in range(BPC):
                        for c in range(2):
                            nc.tensor.matmul(
                                ps_sl[:, b : b + 1],
                                langt2[:, c, b, :],
                                wls[:, c : c + 1],
                                start=(c == 0),
                                stop=(c == 1),
                            )
                    nc.scalar.activation(e_sb, ps_sl, AF.Exp)

                def emit_attention_b():
                    # em (exp * mask) lives here, not in part A: on the
                    # in-order DVE queue it would otherwise block tile 0's
                    # h0 behind the exp latency
                    nc.vector.tensor_mul(em_sb, e_sb, maskt)
                    # denom per batch + reciprocal (the per-batch scale cancels
                    # in the output; it only keeps fp16 magnitudes in range)
                    ps_dn = pmv.tile([1, BPC], F32, tag="mv")
                    nc.tensor.matmul(ps_dn, ones_m, em_sb, start=True, stop=True)
                    nc.vector.reciprocal(rd32, ps_dn)
                    nc.vector.tensor_copy(rdf, rd32)
                    # --- attention part B: af matmuls + scaling.  Emitted
                    # after tile 0's L1 so its PE/DVE work fills the
                    # exp->em dependency bubble instead of stalling the head.
                    # psum tiles borrow the po0/po1 tags: they are dead by the
                    # time tile 0's L2 needs the banks, and ph0/ph1 rotation
                    # stays clean for tiles 1..2.
                    ps_rdb = pop0.tile([128, BPC], F32, tag="po0")
                    nc.tensor.matmul(ps_rdb, ones_1x128, rdf, start=True, stop=True)
                    rdb = cp.tile([128, BPC], F32)
                    nc.vector.tensor_copy(rdb, ps_rdb)
                    ps_af = pop1.tile([128, 2, BPC], F32, tag="po1")
                    for b in range(BPC):
                        for c in range(2):
                            nc.tensor.matmul(
                                ps_af[:, c, b : b + 1],
                                langm[:, b, c * 128 : (c + 1) * 128],
                                em_sb[:, b : b + 1],
                                start=True,
                                stop=True,
                            )
                    for c in range(2):
                        nc.vector.tensor_mul(af[:, c, :], ps_af[:, c, :], rdb)
                    nc.vector.tensor_mul(b4af, af, b4rep)

                mv_c1_pending = []

                def _mv_group(pr, c, xsa, xqa, xsb, xqb):
                    # one K-chunk of BOTH tiles' matvecs as a 4-way
                    # concurrent PE column-group issue (measured: 4 groups
                    # with different moving streams issue within ~16ns and
                    # overlap fully -> 1 slot instead of 2):
                    #   tile 2p:   dot -> partitions 0-31,  ss -> 64-95
                    #   tile 2p+1: dot -> partitions 32-63, ss -> 96-127
                    # skip_group_check: the sim's zero-region tracker maps
                    # partition offsets to overlapping phantom rows for
                    # 32-wide column-group outputs; the real groups are
                    # partition-disjoint.
                    st = {"start": c == 0, "stop": c == 1,
                          "skip_group_check": True}
                    nc.tensor.matmul(
                        pr[0:32, :], ws32[:, c, :], xsa[c],
                        tile_position=(0, 0), **st,
                    )
                    nc.tensor.matmul(
                        pr[32:64, :], ws32[:, c, :], xsb[c],
                        tile_position=(0, 32), **st,
                    )
                    nc.tensor.matmul(
                        pr[64:96, :], ones32, xqa[c],
                        tile_position=(0, 64), **st,
                    )
                    nc.tensor.matmul(
                        pr[96:128, :], ones32, xqb[c],
                        tile_position=(0, 96), **st,
                    )

                def emit_mv_c0(a, b):
                    # first K-chunk in this iteration; second chunk + rsqrt
                    # deferred to the next -> every iteration carries exactly
                    # one 4-way mv slot (10 PE slots flat)
                    ta, xsa, xqa = a
                    tb, xsb, xqb = b
                    p = ta // 2
                    pr = pmv.tile([128, TILE], F32, tag="mv", name=f"pair{p}")
                    _mv_group(pr, 0, xsa, xqa, xsb, xqb)
                    mv_c1_pending.append((p, pr, xsa, xqa, xsb, xqb))

                def emit_mv_c1():
                    p, pr, xsa, xqa, xsb, xqb = mv_c1_pending.pop(0)
                    _mv_group(pr, 1, xsa, xqa, xsb, xqb)
                    if True:
                        # batched epilogue for the pair: one rsqrt over
                        # partitions 64..96 (both ss rows; ACT cost is
                        # partition-count independent), one multiply over
                        # partitions 0..32 (both dot rows).  Unwritten
                        # partitions in between produce garbage lanes that
                        # are never stored.  The multiply is deferred to the
                        # FRONT of the next iteration's DVE queue: it balances
                        # the per-iteration DVE load (which otherwise exceeds
                        # the PE period on epilogue iterations) while still
                        # completing before the pair bank's WAR reuse.
                        rc = epp.tile([33, TILE], F32, tag="rc")
                        inst = nc.scalar.activation(
                            rc, pr[64:97, :], AF.Sqrt, bias=eps_sb
                        )
                        inst.ins.func = AF.Rsqrt
                        epi_pending.append((p, pr, rc))

                pending = []
                xq1_pending = []
                epi_pending = []

                def emit_epilogue():
                    p, pr, rc = epi_pending.pop(0)
                    nc.vector.tensor_mul(out2[:, p, :], pr[0:33, :], rc)
                    if p == NPAIR // 2 - 1:
                        # first half of the output: overlap store
                        nc.sync.dma_start(
                            out=outv[:, 0, 0 : NPAIR // 2, :],
                            in_=out2[0:1, 0 : NPAIR // 2, :],
                        )
                        nc.sync.dma_start(
                            out=outv[:, 1, 0 : NPAIR // 2, :],
                            in_=out2[32:33, 0 : NPAIR // 2, :],
                        )
                xtv2 = xt_d[0:256, :].rearrange(
                    "(c p) (bb r) -> p c bb r", c=2, bb=BPC
                )

                # --- batch-0/1 loads.  Queue priorities (Q0=gpsimd,
                # Q10=scalar, Q1=sync; the 16 DMA engines round-robin across
                # queues, so the critical fill tensors must not sit behind
                # bulk x):
                #   gpsimd: langt2 FIRST (gates attention A), x first halves
                #           (tile 0's L1), then x second halves (tile 1)
                #   scalar: consts; batch-1 x is issued AFTER attention A's
                #           exp in the in-order scalar stream, so it starts
                #           flowing only once the critical fill has landed
                #   sync:   w3t (gates L1), langm (attention B), w4t (L2),
                #           then the bb>=2 batch loads (throttled by xp bufs)
                x01_0 = xp.tile([128, 2, N], F16, tag="x01", name="x01_0", bufs=3)
                x2_0 = xp.tile([64, N], F16, tag="x2", name="x2_0", bufs=3)
                x01_1 = xp.tile([128, 2, N], F16, tag="x01", name="x01_1", bufs=3)
                x2_1 = xp.tile([64, N], F16, tag="x2", name="x2_1", bufs=3)
                # Fill transfers: one deadline-ordered FIFO on sync, with
                # langt2 alone on the gpsimd queue in parallel.  (Spreading
                # descriptors over 3 queues accelerates tiles 0-1 but starves
                # the batch stream for tiles 2-6: net +8us.  The sync engine's
                # ~700ns-per-descriptor issue rate does leave 103-181 GB/s
                # dips between small descriptors, but every rebalance tried
                # costs more mid-ramp than it saves up front.)
                nc.sync.dma_start(out=w3t, in_=w3_d.rearrange("p (c d) -> p c d", c=2))
                nc.gpsimd.dma_start(
                    out=langt2, in_=langt2_d.rearrange("p (c bm) -> p c bm", c=2)
                )
                nc.sync.dma_start(out=x01_0[:, 0, 0:TILE], in_=xtv2[:, 0, 0, 0:TILE])
                nc.sync.dma_start(out=x01_0[:, 1, 0:TILE], in_=xtv2[:, 1, 0, 0:TILE])
                nc.sync.dma_start(out=x2_0[:, 0:TILE], in_=xtv[256:320, 0, 0:TILE])
                nc.sync.dma_start(out=w3c2, in_=w3c_d)
                nc.sync.dma_start(out=x01_0[:, 0, TILE:N], in_=xtv2[:, 0, 0, TILE:N])
                nc.sync.dma_start(out=x01_0[:, 1, TILE:N], in_=xtv2[:, 1, 0, TILE:N])
                nc.sync.dma_start(out=x2_0[:, TILE:N], in_=xtv[256:320, 0, TILE:N])
                nc.sync.dma_start(out=langm, in_=langm_d)
                nc.sync.dma_start(out=w4t, in_=w4_d.rearrange("p (c d) -> p c d", c=2))
                nc.sync.dma_start(out=x01_1, in_=xtv2[:, :, 1])
                nc.sync.dma_start(out=x2_1, in_=xtv[256:320, 1])

                # Dummy matmuls to flip the HAM clock gate (~3.4us of sustained
                # activity) while the first DMAs land.
                for wi in range(N_WARM):
                    pw = php0.tile([128, TILE], F32, tag="ph0", name=f"pw{wi}")
                    nc.tensor.matmul(
                        pw, warm[:, 0:128], warm, start=True, stop=True
                    )

                xbatches = {0: (x01_0, x2_0), 1: (x01_1, x2_1)}
                # batches 2-3 queue behind the fill-critical transfers on
                # sync (fresh pool slots, no WAR gate): the per-engine queue
                # FIFO starts them only after the fill-critical set
                for nbb in (2, 3):
                    x01n = xp.tile([128, 2, N], F16, tag="x01", bufs=3,
                                   name=f"x01_{nbb}")
                    x2n = xp.tile([64, N], F16, tag="x2", bufs=3,
                                  name=f"x2_{nbb}")
                    nc.sync.dma_start(out=x01n, in_=xtv2[:, :, nbb])
                    nc.sync.dma_start(out=x2n, in_=xtv[256:320, nbb])
                    xbatches[nbb] = (x01n, x2n)
                phs = {}

                def emit_l1(t):
                    # L1 stream-major: consecutive matmuls reuse the same
                    # moving tensor (x0 twice, x1 twice, x2 pair) -- a
                    # moving-stream switch costs ~85ns of PE issue time.
                    x01, x2 = xbatches[t // 2]
                    x0 = x01[:, 0, :]
                    x1 = x01[:, 1, :]
                    rs = slice((t % 2) * TILE, (t % 2 + 1) * TILE)
                    ph0 = php0.tile([128, TILE], F32, tag="ph0", name=f"ph0_{t}")
                    ph1 = php1.tile([128, TILE], F32, tag="ph1", name=f"ph1_{t}")
                    nc.tensor.matmul(
                        ph0, w3t[:, 0, 0:128], x0[:, rs], start=True, stop=False
                    )
                    nc.tensor.matmul(
                        ph1, w3t[:, 0, 128:256], x0[:, rs], start=True, stop=False
                    )
                    nc.tensor.matmul(
                        ph0, w3t[:, 1, 0:128], x1[:, rs], start=False, stop=False
                    )
                    nc.tensor.matmul(
                        ph1, w3t[:, 1, 128:256], x1[:, rs], start=False, stop=False
                    )
                    # the two K=32 tail matmuls sit in different PE
                    # row-groups (rows 0-31 / 32-63) and run concurrently
                    nc.tensor.matmul(
                        ph0, w3c2[0:32, :], x2[0:32, rs], start=False, stop=True
                    )
                    nc.tensor.matmul(
                        ph1, w3c2[32:64, :], x2[32:64, rs], start=False, stop=True
                    )
                    phs[t] = (ph0, ph1)

                # attention A's 16 small matmuls + exp run during the tail of
                # the x/weight DMAs, before tile 0's L1
                emit_attention_a()
                # bridge dummies keep the PE busy over the exp -> em -> attB
                # latency (an idle PE re-arms the HAM gate: ~10us half clock);
                # on the mv tag they also WAR-wait for the exp read of ps_sl,
                # which places them exactly in that hole
                for wi in range(N_BRIDGE):
                    pwb = pmv.tile([128, TILE], F32, tag="mv", name=f"pwb{wi}")
                    nc.tensor.matmul(
                        pwb, warm[:, 0:128], warm, start=True, stop=True
                    )
                # software pipeline: tile t+1's L1 is emitted BEFORE tile t's
                # h/L2, so the PE chews next tile's L1 while DVE/ACT turn
                # around h0/h1 -- the h0 latency leaves the critical cycle
                emit_l1(0)
                for t in range(NT):
                    bb = t // 2
                    if t % 2 == 1 and 4 <= (t + 3) // 2 < BPC:
                        # batch prefetch ahead of first use; with bufs=3 the
                        # WAR gate gives a ~3-tile lead
                        nbb = (t + 3) // 2
                        x01n = xp.tile([128, 2, N], F16, tag="x01", bufs=3,
                                       name=f"x01_{nbb}")
                        x2n = xp.tile([64, N], F16, tag="x2", bufs=3,
                                      name=f"x2_{nbb}")
                        nc.sync.dma_start(out=x01n, in_=xtv2[:, :, nbb])
                        nc.sync.dma_start(out=x2n, in_=xtv[256:320, nbb])
                        xbatches[nbb] = (x01n, x2n)
                    if t + 1 < NT:
                        emit_l1(t + 1)
                    if epi_pending:
                        # pair out-multiply at the DVE queue front: done
                        # (~0.7us in) before this iteration's mv needs the
                        # pair bank, and it fills the iteration whose h1
                        # rides ACT
                        emit_epilogue()
                    ph0, ph1 = phs.pop(t)
                    # h0 on DVE; h1 alternates DVE (even) / ACT (odd): with
                    # the deferred out-multiply this keeps every engine's
                    # per-iteration load ~2.1-2.4us, under the PE's ~2.7
                    h0 = wp.tile([128, TILE], F16, tag="h0")
                    nc.vector.tensor_scalar(
                        out=h0, in0=ph0,
                        scalar1=b3s[:, 0:1], scalar2=0.0,
                        op0=mybir.AluOpType.add, op1=mybir.AluOpType.max,
                    )
                    h1 = wp.tile([128, TILE], F16, tag="h1")
                    if t % 2 == 1 or t < 4:
                        nc.vector.tensor_scalar(
                            out=h1, in0=ph1,
                            scalar1=b3s[:, 1:2], scalar2=0.0,
                            op0=mybir.AluOpType.add, op1=mybir.AluOpType.max,
                        )
                    else:
                        nc.scalar.activation(
                            h1, ph1, AF.Relu, bias=b3s[:, 1:2]
                        )
                    if len(xq1_pending) > XQ_LAG - 1:
                        # both squares of tile t-XQ_LAG as ONE merged
                        # [128, 2*TILE] fp16 DVE op (2x mode): the xv
                        # input is guaranteed done, so the strict-FIFO
                        # DVE queue never stalls waiting on ACT.
                        nc.vector.tensor_mul(*xq1_pending.pop(0))
                    if t == 0:
                        # attention part B's 17 matmuls fill the PE gap
                        # while DVE computes h0/h1; af is then ready well
                        # before xv(0) needs it
                        emit_attention_b()
                    if len(pending) > 2 and pending[0][0] % 2 == 0:
                        # both tiles of a completed pair: 8 matvec matmuls
                        # as two 4-way concurrent column-group slots.  Both
                        # K-chunks stay in ONE iteration: an accumulation
                        # group left open across interleaved L1/L2 matmuls
                        # costs ~500ns/tile (measured).
                        emit_mv_c0(pending.pop(0), pending.pop(0))
                        emit_mv_c1()
                    # L2 ordered h0-first so it can start before h1 is done
                    po0 = pop0.tile([128, TILE], F32, tag="po0")
                    po1 = pop1.tile([128, TILE], F32, tag="po1")
                    nc.tensor.matmul(
                        po0, w4t[:, 0, 0:128], h0, start=True, stop=False
                    )
                    nc.tensor.matmul(
                        po1, w4t[:, 0, 128:256], h0, start=True, stop=False
                    )
                    nc.tensor.matmul(
                        po0, w4t[:, 1, 0:128], h1, start=False, stop=True
                    )
                    nc.tensor.matmul(
                        po1, w4t[:, 1, 128:256], h1, start=False, stop=True
                    )
                    xvt = wp.tile([128, 2, TILE], F16, tag="xv")
                    for o, po in ((0, po0), (1, po1)):
                        nc.scalar.activation(
                            xvt[:, o, :], po, AF.Relu,
                            bias=b4af[:, o, bb : bb + 1],
                            scale=af[:, o, bb : bb + 1],
                        )
                    xqt = wp.tile([128, 2, TILE], F16, tag="xq")
                    xq1_pending.append((xqt, xvt, xvt))
                    pending.append((t, [xvt[:, 0, :], xvt[:, 1, :]],
                                    [xqt[:, 0, :], xqt[:, 1, :]]))
                for q in xq1_pending:
                    nc.vector.tensor_mul(*q)
                xq1_pending.clear()
                while pending or mv_c1_pending:
                    if mv_c1_pending:
                        emit_mv_c1()
                    else:
                        emit_mv_c0(pending.pop(0), pending.pop(0))
                    while epi_pending:
                        emit_epilogue()
                nc.sync.dma_start(
                    out=outv[:, 0, NPAIR // 2 : NPAIR, :],
                    in_=out2[0:1, NPAIR // 2 : NPAIR, :],
                )
                nc.sync.dma_start(
                    out=outv[:, 1, NPAIR // 2 : NPAIR, :],
                    in_=out2[32:33, NPAIR // 2 : NPAIR, :],
                )
    nc.compile()
    return nc


_NC_CACHE = {}


def _get_nc():
    if "nc" not in _NC_CACHE:
        _NC_CACHE["nc"] = _build_nc()
    return _NC_CACHE["nc"]


def _f16(x):
    return np.ascontiguousarray(x).astype(np.float16)


def kernel(**inputs) -> np.ndarray:
    object_feat = np.ascontiguousarray(np.asarray(inputs["object_feat"], np.float32))
    lang_feat = np.ascontiguousarray(np.asarray(inputs["lang_feat"], np.float32))
    lang_mask = np.asarray(inputs["lang_mask"])
    W3 = np.asarray(inputs["W3"], np.float32)
    W4 = np.asarray(inputs["W4"], np.float32)
    b3 = np.asarray(inputs["b3"], np.float32)
    b4 = np.asarray(inputs["b4"], np.float32)
    Wa = np.asarray(inputs["Wa"], np.float32)
    Ws = np.asarray(inputs["Ws"], np.float32)
    bs = np.asarray(inputs["bs"], np.float32)

    w3r = _f16(W3[0:256].reshape(2, 128, D).transpose(1, 0, 2).reshape(128, 2 * D))
    w3c2 = np.concatenate([W3[256:288, 0:128], W3[256:288, 128:256]], axis=0)
    w3cr = _f16(w3c2)
    w4r = _f16(W4.reshape(2, 128, D).transpose(1, 0, 2).reshape(128, 2 * D))
    csth = np.zeros((128, 4), np.float16)
    csth[:, 0:2] = _f16(Ws[:, 0].reshape(2, 128).T)
    csth[:, 2:4] = _f16(Wa[D:, 0].reshape(2, 128).T)

    in_maps = []
    for i in range(NCORES):
        sl = slice(i * BPC, (i + 1) * BPC)
        of = object_feat[sl]                                   # (BPC, N, C1)
        lf = lang_feat[sl]                                     # (BPC, M, D)
        xt = of.reshape(R, C1).T
        xt_dup = np.concatenate([xt, xt[256:288]], axis=0)     # (320, R)
        cstf = np.zeros((128, 29), np.float32)
        cstf[:, 0:2] = b3.reshape(2, 128).T
        cstf[:, 2:4] = b4.reshape(2, 128).T
        cstf[0, 4] = bs[0]
        cstf[0:M, 5:13] = lang_mask[sl].T.astype(np.float32)
        # b4 chunk-major, replicated across the BPC batches: [128, 2, 8]
        cstf[:, 13:29] = np.repeat(
            b4.reshape(2, 128).T[:, :, None], BPC, axis=2
        ).reshape(128, 2 * BPC)
        lt2 = lf.transpose(2, 0, 1).reshape(2, 128, BPC * M)
        in_maps.append(
            {
                "xt": _f16(xt_dup),
                "langm": _f16(lf.transpose(1, 0, 2)),
                "langt2": _f16(lt2.transpose(1, 0, 2).reshape(128, 2 * BPC * M)),
                "cstf": cstf,
                "csth": csth,
                "w3": w3r,
                "w3c": w3cr,
                "w4": w4r,
            }
        )

    nc = _get_nc()
    res = run_bass_kernel_spmd(nc, in_maps, core_ids=list(range(NCORES)))
    _NC_CACHE["last_results"] = res
    out = np.empty((B, 1, N), np.float32)
    for i in range(NCORES):
        out[i * BPC : (i + 1) * BPC, 0, :] = res.results[i]["out"].reshape(BPC, N)
    out += bs[0]  # final bias applied on host (constant add)
    return out


# revision 88
# speedup vs baseline: 1.1507x; 1.0040x over previous
"""Trainium2 Bass kernel for nn_AttenModule (B=64, N=1024, M=80, C1=288, D=256).

Math notes (derived from the reference):
  score[b,n,m] = (oa@w_o)[b,n] + (lang@w_l)[b,m] + ba, softmax over m.
  The (oa@w_o)[b,n] and ba terms are constant along m, so they cancel in the
  softmax -> att[b,n,:] == softmax_m(mask(lang[b]@w_l)) is independent of n,
  and att_feat[b,:] = sum_m att[b,m]*lang[b,m,:] is a per-batch vector.
  Hence the entire W1/W2/w_o branch is dead.

  out = (v@Ws)/max(||v||,eps) + bs with v = relu(osc * af[b]) is
  scale-invariant in v (relu commutes with positive scales), so the softmax
  denominator only needs enough accuracy to keep fp16 ranges in check.

  Remaining per-row work (row = (b,n)):
    osc = relu(x @ W3 + b3) @ W4 + b4            # x = object_feat row (288,)
    v   = relu(osc * af[b])                      # af[b] = softmax(lang@w_l) @ lang
    out = (v @ Ws) / sqrt(||v||^2 + 1e-24) + bs

Device layout: feature-on-partition (transposed activations).  Per core
(8 cores, data-parallel over B): 8 batches = 8192 rows, row-tiles of 512.
All matmuls run in fp16 (11-bit mantissa, 1 cycle/row); PSUM accumulates
fp32.  Biases and the attention scale vector stay fp32 (applied via the
ACT engine's per-partition scale/bias).

Scheduling notes (v2):
  - Software pipelining: tile t+1's L1 matmuls are emitted before tile t's
    h/L2, so the PE runs next tile's L1 while DVE turns around h0/h1 and
    the h0 latency leaves the critical cycle (steady period ~2.7us/tile,
    PE-bound at ~11 matmul slots).
  - Engine split per tile: DVE h0/h1 (+merged xq at 1-tile lag, pair
    out-mul); ACT xv0/xv1 (+pair rsqrt); ACT keeps ~0.8us/tile of slack to
    drain its pipe-fill backlog (exp + early xv bursts).
  - Epilogues are batched per 2 tiles ("pair"): tile 2p's dot/ss go to PSUM
    partitions 0-31/64-95 of one bank (32-wide PE column groups), tile
    2p+1's to 32-63/96-127.  One ACT rsqrt over partitions 64..96 and one
    DVE multiply over 0..32 then serve both tiles (ACT/DVE cost is
    partition-count independent).  Matvecs trail the main loop by MV_LAG
    tiles so the ACT-produced xv/xq are always ready.
  - attention is split: scores+exp before the loop (PE/ACT), the
    mask/denominator/af chain inside tile 0 (where its PE matmuls fill the
    exp->em dependency bubble); psum tiles borrow the po0/po1 tags so the
    ph0/ph1 double-buffer rotation stays clean across tiles 0-2.
  - DMA: the 16 engines round-robin all non-empty HWDGE queues with no
    priorities, one queue sustains only ~110-180 GB/s, each dma_start costs
    ~700ns of serial issue time on its engine, and reused semaphores force
    issue-order waits.  So: the fill rides one deadline-ordered FIFO on
    sync (weights, batch-0 x in halves, langm, then batches 1-3), langt2
    alone on gpsimd, and batches 4-7 are emitted mid-loop where the xp-pool
    WAR gate self-paces them ~3 tiles ahead of use.
  - The HAM clock gate holds the PE at 1.2 GHz until ~3.4us of sustained
    activity and re-arms on any >~0.8us idle (costing ~10us of half clock):
    N_WARM dummy matmuls cover the first x/weight DMAs.  (Bridge dummies
    are gone: mv-tagged they WAR-blocked L1(0) behind the exp read of
    ps_sl; L1(0)+L1(1) themselves cover the exp->em hole.)
"""

import numpy as np

import concourse.bacc as bacc
import concourse.tile as tile
from concourse import mybir
from concourse.bass_utils import run_bass_kernel_spmd

B, N, M = 64, 1024, 80
C1, D = 288, 256
NCORES = 8
BPC = B // NCORES          # batches per core
R = BPC * N                # rows per core
TILE = 512
NT = R // TILE             # row tiles per core
NPAIR = NT // 2
F32 = mybir.dt.float32
F16 = mybir.dt.float16

N_WARM = 8                 # initial PE warm-up matmuls (HAM clock gate)
N_BRIDGE = 0               # bridge warmups over the attention exp->denom gap
XQ_LAG = 1                 # tiles of lag for the merged xq = xv*xv DVE op
MV_LAG = 3                 # tiles of lag for the matvec + epilogue


def _build_nc():
    nc = bacc.Bacc("TRN2", target_bir_lowering=False, debug=False)

    # xt rows 0..287 = x^T; rows 288..319 duplicate rows 256..287 so the two
    # K=32 tail matmuls (one per out-chunk) can run in concurrent PE row-groups
    xt_d = nc.dram_tensor("xt", [C1 + 32, R], F16, kind="ExternalInput").ap()
    langm_d = nc.dram_tensor("langm", [M, BPC, D], F16, kind="ExternalInput").ap()
    # langt2 pre-arranged as [128, 2, BPC*M] on the host
    langt2_d = nc.dram_tensor("langt2", [128, 2 * BPC * M], F16, kind="ExternalInput").ap()
    # w3/w4 pre-arranged as [128, 2, D] (chunk-major)
    w3_d = nc.dram_tensor("w3", [128, 2 * D], F16, kind="ExternalInput").ap()
    # w3c2: rows 0-31 = W3[256:288, 0:128], rows 32-63 = W3[256:288, 128:256]
    w3c_d = nc.dram_tensor("w3c", [64, 128], F16, kind="ExternalInput").ap()
    w4_d = nc.dram_tensor("w4", [128, 2 * D], F16, kind="ExternalInput").ap()
    # packed fp32 consts: cols [b3(2) | b4(2) | bs(1) | maskt(8, rows 0-79) |
    #                           b4rep(16 = b4 chunk-major replicated per batch)]
    cstf_d = nc.dram_tensor("cstf", [128, 29], F32, kind="ExternalInput").ap()
    # packed fp16 consts: cols [ws(2) | wl(2)]
    csth_d = nc.dram_tensor("csth", [128, 4], F16, kind="ExternalInput").ap()
    out_d = nc.dram_tensor("out", [1, R], F32, kind="ExternalOutput").ap()

    AF = mybir.ActivationFunctionType

    with tile.TileContext(nc) as tc:
        with tc.tile_pool(name="const", bufs=1) as cp:
            # csth/cstf at the HEAD of the sync queue: tiny, and csth feeds
            # the ws32 copies + attention scores
            csth = cp.tile([128, 4], F16)
            nc.sync.dma_start(out=csth, in_=csth_d)
            cstf = cp.tile([128, 29], F32)
            nc.sync.dma_start(out=cstf, in_=cstf_d)
            w3t = cp.tile([128, 2, D], F16)
            w3c2 = cp.tile([64, 128], F16)
            langt2 = cp.tile([128, 2, BPC, M], F16)
            w4t = cp.tile([128, 2, D], F16)
            langm = cp.tile([M, BPC, D], F16)
            wss = csth[:, 0:2]
            wls = csth[:, 2:4]
            b3s = cstf[:, 0:2]
            b4s = cstf[:, 2:4]
            maskt = cstf[0:M, 5:13]
            b4rep = cstf[:, 13:29].rearrange("p (c b) -> p c b", c=2)
            ones_m = cp.tile([M, 1], F16)
            nc.vector.memset(ones_m, 1.0)
            ones_1x128 = cp.tile([1, 128], F16)
            nc.vector.memset(ones_1x128, 1.0)
            ones_128 = cp.tile([128, 1], F16)
            nc.vector.memset(ones_128, 1.0)
            # 32-wide stationaries for the matvecs (col 0 live, rest zero):
            # each dot/ss matmul then fills a whole 32-partition PE column
            # group, so every psum partition the batched epilogue reads is
            # written (same PE cost -- it scales with the moving free size)
            ws32 = cp.tile([128, 2, 32], F16)
            nc.vector.memset(ws32, 0.0)
            nc.vector.tensor_copy(ws32[:, 0, 0:1], wss[:, 0:1])
            nc.vector.tensor_copy(ws32[:, 1, 0:1], wss[:, 1:2])
            ones32 = cp.tile([128, 32], F16)
            nc.vector.memset(ones32, 0.0)
            nc.vector.memset(ones32[:, 0:1], 1.0)
            eps_sb = cp.tile([33, 1], F32)
            nc.vector.memset(eps_sb, 1e-24)
            # PE warm-up: the HAM clock gate keeps the PE at 1.2 GHz until it
            # sees ~3.4us of sustained activity, and re-throttles (costing
            # ~10us of half-speed) if the PE goes idle again.
            warm = cp.tile([128, TILE], F16)
            nc.gpsimd.memset(warm, 0.0)

            # ---------- main loop (attention interleaved into it) ----------
            with (
                # x tiles use bufs=3 (per-tag): the WAR gate then self-paces
                # batch bb's DMA ~3 tiles ahead of its first use
                tc.tile_pool(name="xt", bufs=3) as xp,
                tc.tile_pool(name="work", bufs=4) as wp,
                tc.tile_pool(name="ep", bufs=3) as epp,
                tc.tile_pool(name="ph0p", bufs=2, space="PSUM") as php0,
                tc.tile_pool(name="ph1p", bufs=2, space="PSUM") as php1,
                tc.tile_pool(name="po0p", bufs=2, space="PSUM") as pop0,
                tc.tile_pool(name="po1p", bufs=1, space="PSUM") as pop1,
                tc.tile_pool(name="pmv", bufs=1, space="PSUM") as pmv,
            ):
                # out2[0, p, :] = tile 2p's outputs, out2[32, p, :] = tile 2p+1's
                out2 = cp.tile([33, NPAIR, TILE], F32)
                # DRAM view [1, 2, NPAIR, TILE]: index [o, k, p, r] = tile 2p+k
                outv = out_d.rearrange("o (p k r) -> o k p r", k=2, r=TILE)
                xtv = xt_d.rearrange("c (bb r) -> c bb r", bb=BPC)

                e_sb = cp.tile([M, BPC], F32)
                em_sb = cp.tile([M, BPC], F16)
                rd32 = cp.tile([1, BPC], F32)
                rdf = cp.tile([1, BPC], F16)
                af = cp.tile([128, 2, BPC], F32)
                b4af = cp.tile([128, 2, BPC], F32)

                def emit_attention_a():
                    # --- attention part A: scores, exp, mask.  The denom
                    # matmul lives in part B: here it would block the PE
                    # queue on the exp->em latency right when tile 0's L1
                    # could run.
                    ps_sl = pmv.tile([M, BPC], F32, tag="mv")
                    for b in range(BPC):
                        for c in range(2):
                            nc.tensor.matmul(
                                ps_sl[:, b : b + 1],
                                langt2[:, c, b, :],
                                wls[:, c : c + 1],
                                start=(c == 0),
                                stop=(c == 1),
                            )
                    nc.scalar.activation(e_sb, ps_sl, AF.Exp)

                def emit_attention_b():
                    # em (exp * mask) lives here, not in part A: on the
                    # in-order DVE queue it would otherwise block tile 0's
                    # h0 behind the exp latency
                    nc.vector.tensor_mul(em_sb, e_sb, maskt)
                    # denom per batch + reciprocal (the per-batch scale cancels
                    # in the output; it only keeps fp16 magnitudes in range)
                    ps_dn = pmv.tile([1, BPC], F32, tag="mv")
                    nc.tensor.matmul(ps_dn, ones_m, em_sb, start=True, stop=True)
                    nc.vector.reciprocal(rd32, ps_dn)
                    nc.vector.tensor_copy(rdf, rd32)
                    # --- attention part B: af matmuls + scaling.  Emitted
                    # after tile 0's L1 so its PE/DVE work fills the
                    # exp->em dependency bubble instead of stalling the head.
                    # psum tiles borrow the po0/po1 tags: they are dead by the
                    # time tile 0's L2 needs the banks, and ph0/ph1 rotation
                    # stays clean for tiles 1..2.
                    ps_rdb = pop0.tile([128, BPC], F32, tag="po0")
                    nc.tensor.matmul(ps_rdb, ones_1x128, rdf, start=True, stop=True)
                    rdb = cp.tile([128, BPC], F32)
                    nc.vector.tensor_copy(rdb, ps_rdb)
                    ps_af = pop1.tile([128, 2, BPC], F32, tag="po1")
                    for b in range(BPC):
                        for c in range(2):
                            nc.tensor.matmul(
                                ps_af[:, c, b : b + 1],
                                langm[:, b, c * 128 : (c + 1) * 128],
                                em_sb[:, b : b + 1],
                                start=True,
                                stop=True,
                            )
                    for c in range(2):
                        nc.vector.tensor_mul(af[:, c, :], ps_af[:, c, :], rdb)
                    nc.vector.tensor_mul(b4af, af, b4rep)

                mv_c1_pending = []

                def _mv_group(pr, c, xsa, xqa, xsb, xqb):
                    # one K-chunk of BOTH tiles' matvecs as a 4-way
                    # concurrent PE column-group issue (measured: 4 groups
                    # with different moving streams issue within ~16ns and
                    # overlap fully -> 1 slot instead of 2):
                    #   tile 2p:   dot -> partitions 0-31,  ss -> 64-95
                    #   tile 2p+1: dot -> partitions 32-63, ss -> 96-127
                    # skip_group_check: the sim's zero-region tracker maps
                    # partition offsets to overlapping phantom rows for
                    # 32-wide column-group outputs; the real groups are
                    # partition-disjoint.
                    st = {"start": c == 0, "stop": c == 1,
                          "skip_group_check": True}
                    nc.tensor.matmul(
                        pr[0:32, :], ws32[:, c, :], xsa[c],
                        tile_position=(0, 0), **st,
                    )
                    nc.tensor.matmul(
                        pr[32:64, :], ws32[:, c, :], xsb[c],
                        tile_position=(0, 32), **st,
                    )
                    nc.tensor.matmul(
                        pr[64:96, :], ones32, xqa[c],
                        tile_position=(0, 64), **st,
                    )
                    nc.tensor.matmul(
                        pr[96:128, :], ones32, xqb[c],
                        tile_position=(0, 96), **st,
                    )

                def emit_mv_c0(a, b):
                    # first K-chunk in this iteration; second chunk + rsqrt
                    # deferred to the next -> every iteration carries exactly
                    # one 4-way mv slot (10 PE slots flat)
                    ta, xsa, xqa = a
                    tb, xsb, xqb = b
                    p = ta // 2
                    pr = pmv.tile([128, TILE], F32, tag="mv", name=f"pair{p}")
                    _mv_group(pr, 0, xsa, xqa, xsb, xqb)
                    mv_c1_pending.append((p, pr, xsa, xqa, xsb, xqb))

                def emit_mv_c1():
                    p, pr, xsa, xqa, xsb, xqb = mv_c1_pending.pop(0)
                    _mv_group(pr, 1, xsa, xqa, xsb, xqb)
                    if True:
                        # batched epilogue for the pair: one rsqrt over
                        # partitions 64..96 (both ss rows; ACT cost is
                        # partition-count independent), one multiply over
                        # partitions 0..32 (both dot rows).  Unwritten
                        # partitions in between produce garbage lanes that
                        # are never stored.  The multiply is deferred to the
                        # FRONT of the next iteration's DVE queue: it balances
                        # the per-iteration DVE load (which otherwise exceeds
                        # the PE period on epilogue iterations) while still
                        # completing before the pair bank's WAR reuse.
                        rc = epp.tile([33, TILE], F32, tag="rc")
                        inst = nc.scalar.activation(
                            rc, pr[64:97, :], AF.Sqrt, bias=eps_sb
                        )
                        inst.ins.func = AF.Rsqrt
                        epi_pending.append((p, pr, rc))

                pending = []
                xq1_pending = []
                epi_pending = []

                def emit_epilogue():
                    p, pr, rc = epi_pending.pop(0)
                    nc.vector.tensor_mul(out2[:, p, :], pr[0:33, :], rc)
                    if p == NPAIR // 2 - 1:
                        # first half of the output: overlap store
                        nc.sync.dma_start(
                            out=outv[:, 0, 0 : NPAIR // 2, :],
                            in_=out2[0:1, 0 : NPAIR // 2, :],
                        )
                        nc.sync.dma_start(
                            out=outv[:, 1, 0 : NPAIR // 2, :],
                            in_=out2[32:33, 0 : NPAIR // 2, :],
                        )
                xtv2 = xt_d[0:256, :].rearrange(
                    "(c p) (bb r) -> p c bb r", c=2, bb=BPC
                )

                # --- batch-0/1 loads.  Queue priorities (Q0=gpsimd,
                # Q10=scalar, Q1=sync; the 16 DMA engines round-robin across
                # queues, so the critical fill tensors must not sit behind
                # bulk x):
                #   gpsimd: langt2 FIRST (gates attention A), x first halves
                #           (tile 0's L1), then x second halves (tile 1)
                #   scalar: consts; batch-1 x is issued AFTER attention A's
                #           exp in the in-order scalar stream, so it starts
                #           flowing only once the critical fill has landed
                #   sync:   w3t (gates L1), langm (attention B), w4t (L2),
                #           then the bb>=2 batch loads (throttled by xp bufs)
                x01_0 = xp.tile([128, 2, N], F16, tag="x01", name="x01_0", bufs=3)
                x2_0 = xp.tile([64, N], F16, tag="x2", name="x2_0", bufs=3)
                x01_1 = xp.tile([128, 2, N], F16, tag="x01", name="x01_1", bufs=3)
                x2_1 = xp.tile([64, N], F16, tag="x2", name="x2_1", bufs=3)
                # Fill transfers: one deadline-ordered FIFO on sync, with
                # langt2 alone on the gpsimd queue in parallel.  (Spreading
                # descriptors over 3 queues accelerates tiles 0-1 but starves
                # the batch stream for tiles 2-6: net +8us.  The sync engine's
                # ~700ns-per-descriptor issue rate does leave 103-181 GB/s
                # dips between small descriptors, but every rebalance tried
                # costs more mid-ramp than it saves up front.)
                nc.sync.dma_start(out=w3t, in_=w3_d.rearrange("p (c d) -> p c d", c=2))
                nc.gpsimd.dma_start(
                    out=langt2, in_=langt2_d.rearrange("p (c bm) -> p c bm", c=2)
                )
                nc.sync.dma_start(out=x01_0[:, 0, 0:TILE], in_=xtv2[:, 0, 0, 0:TILE])
                nc.sync.dma_start(out=x01_0[:, 1, 0:TILE], in_=xtv2[:, 1, 0, 0:TILE])
                nc.sync.dma_start(out=x2_0[:, 0:TILE], in_=xtv[256:320, 0, 0:TILE])
                nc.sync.dma_start(out=w3c2, in_=w3c_d)
                nc.sync.dma_start(out=x01_0[:, 0, TILE:N], in_=xtv2[:, 0, 0, TILE:N])
                nc.sync.dma_start(out=x01_0[:, 1, TILE:N], in_=xtv2[:, 1, 0, TILE:N])
                nc.sync.dma_start(out=x2_0[:, TILE:N], in_=xtv[256:320, 0, TILE:N])
                nc.sync.dma_start(out=langm, in_=langm_d)
                nc.sync.dma_start(out=w4t, in_=w4_d.rearrange("p (c d) -> p c d", c=2))
                nc.sync.dma_start(out=x01_1, in_=xtv2[:, :, 1])
                nc.sync.dma_start(out=x2_1, in_=xtv[256:320, 1])

                # Dummy matmuls to flip the HAM clock gate (~3.4us of sustained
                # activity) while the first DMAs land.
                for wi in range(N_WARM):
                    pw = php0.tile([128, TILE], F32, tag="ph0", name=f"pw{wi}")
                    nc.tensor.matmul(
                        pw, warm[:, 0:128], warm, start=True, stop=True
                    )

                xbatches = {0: (x01_0, x2_0), 1: (x01_1, x2_1)}
                # batches 2-3 queue behind the fill-critical transfers on
                # sync (fresh pool slots, no WAR gate): the per-engine queue
                # FIFO starts them only after the fill-critical set
                for nbb in (2, 3):
                    x01n = xp.tile([128, 2, N], F16, tag="x01", bufs=3,
                                   name=f"x01_{nbb}")
                    x2n = xp.tile([64, N], F16, tag="x2", bufs=3,
                                  name=f"x2_{nbb}")
                    nc.sync.dma_start(out=x01n, in_=xtv2[:, :, nbb])
                    nc.sync.dma_start(out=x2n, in_=xtv[256:320, nbb])
                    xbatches[nbb] = (x01n, x2n)
                phs = {}

                def emit_l1(t):
                    # L1 stream-major: consecutive matmuls reuse the same
                    # moving tensor (x0 twice, x1 twice, x2 pair) -- a
                    # moving-stream switch costs ~85ns of PE issue time.
                    x01, x2 = xbatches[t // 2]
                    x0 = x01[:, 0, :]
                    x1 = x01[:, 1, :]
                    rs = slice((t % 2) * TILE, (t % 2 + 1) * TILE)
                    ph0 = php0.tile([128, TILE], F32, tag="ph0", name=f"ph0_{t}")
                    ph1 = php1.tile([128, TILE], F32, tag="ph1", name=f"ph1_{t}")
                    nc.tensor.matmul(
                        ph0, w3t[:, 0, 0:128], x0[:, rs], start=True, stop=False
                    )
                    nc.tensor.matmul(
                        ph1, w3t[:, 0, 128:256], x0[:, rs], start=True, stop=False
                    )
                    nc.tensor.matmul(
                        ph0, w3t[:, 1, 0:128], x1[:, rs], start=False, stop=False
                    )
                    nc.tensor.matmul(
                        ph1, w3t[:, 1, 128:256], x1[:, rs], start=False, stop=False
                    )
                    # the two K=32 tail matmuls sit in different PE
                    # row-groups (rows 0-31 / 32-63) and run concurrently
                    nc.tensor.matmul(
                        ph0, w3c2[0:32, :], x2[0:32, rs], start=False, stop=True
                    )
                    nc.tensor.matmul(
                        ph1, w3c2[32:64, :], x2[32:64, rs], start=False, stop=True
                    )
                    phs[t] = (ph0, ph1)

                # software pipeline: tile t+1's L1 is emitted BEFORE tile t's
                # h/L2, so the PE chews next tile's L1 while DVE/ACT turn
                # around h0/h1 -- the h0 latency leaves the critical cycle.
                # L1(0) precedes the attention scores: its x first-halves
                # land ~1.3us before langt2, so it is the PE's post-warmup
                # work and the scores get extra margin for langt2 jitter.
                emit_l1(0)
                emit_attention_a()
                for t in range(NT):
                    bb = t // 2
                    if t % 2 == 1 and 4 <= (t + 3) // 2 < BPC:
                        # batch prefetch ahead of first use; with bufs=3 the
                        # WAR gate gives a ~3-tile lead
                        nbb = (t + 3) // 2
                        x01n = xp.tile([128, 2, N], F16, tag="x01", bufs=3,
                                       name=f"x01_{nbb}")
                        x2n = xp.tile([64, N], F16, tag="x2", bufs=3,
                                      name=f"x2_{nbb}")
                        nc.sync.dma_start(out=x01n, in_=xtv2[:, :, nbb])
                        nc.sync.dma_start(out=x2n, in_=xtv[256:320, nbb])
                        xbatches[nbb] = (x01n, x2n)
                    if t + 1 < NT:
                        emit_l1(t + 1)
                    if epi_pending:
                        # pair out-multiply at the DVE queue front: done
                        # (~0.7us in) before this iteration's mv needs the
                        # pair bank, and it fills the iteration whose h1
                        # rides ACT
                        emit_epilogue()
                    ph0, ph1 = phs.pop(t)
                    # h0 on DVE; h1 alternates DVE (even) / ACT (odd): with
                    # the deferred out-multiply this keeps every engine's
                    # per-iteration load ~2.1-2.4us, under the PE's ~2.7
                    h0 = wp.tile([128, TILE], F16, tag="h0")
                    nc.vector.tensor_scalar(
                        out=h0, in0=ph0,
                        scalar1=b3s[:, 0:1], scalar2=0.0,
                        op0=mybir.AluOpType.add, op1=mybir.AluOpType.max,
                    )
                    h1 = wp.tile([128, TILE], F16, tag="h1")
                    if t % 2 == 1 or t < 4:
                        nc.vector.tensor_scalar(
                            out=h1, in0=ph1,
                            scalar1=b3s[:, 1:2], scalar2=0.0,
                            op0=mybir.AluOpType.add, op1=mybir.AluOpType.max,
                        )
                    else:
                        nc.scalar.activation(
                            h1, ph1, AF.Relu, bias=b3s[:, 1:2]
                        )
                    if len(xq1_pending) > XQ_LAG - 1:
                        # both squares of tile t-XQ_LAG as ONE merged
                        # [128, 2*TILE] fp16 DVE op (2x mode): the xv
                        # input is guaranteed done, so the strict-FIFO
                        # DVE queue never stalls waiting on ACT.
                        nc.vector.tensor_mul(*xq1_pending.pop(0))
                    if t == 0:
                        # attention part B's 17 matmuls fill the PE gap
                        # while DVE computes h0/h1; af is then ready well
                        # before xv(0) needs it
                        emit_attention_b()
                    if len(pending) > 2 and pending[0][0] % 2 == 0:
                        # both tiles of a completed pair: 8 matvec matmuls
                        # as two 4-way concurrent column-group slots.  Both
                        # K-chunks stay in ONE iteration: an accumulation
                        # group left open across interleaved L1/L2 matmuls
                        # costs ~500ns/tile (measured).
                        emit_mv_c0(pending.pop(0), pending.pop(0))
                        emit_mv_c1()
                    # L2 ordered h0-first so it can start before h1 is done
                    po0 = pop0.tile([128, TILE], F32, tag="po0")
                    po1 = pop1.tile([128, TILE], F32, tag="po1")
                    nc.tensor.matmul(
                        po0, w4t[:, 0, 0:128], h0, start=True, stop=False
                    )
                    nc.tensor.matmul(
                        po1, w4t[:, 0, 128:256], h0, start=True, stop=False
                    )
                    nc.tensor.matmul(
                        po0, w4t[:, 1, 0:128], h1, start=False, stop=True
                    )
                    nc.tensor.matmul(
                        po1, w4t[:, 1, 128:256], h1, start=False, stop=True
                    )
                    xvt = wp.tile([128, 2, TILE], F16, tag="xv")
                    for o, po in ((0, po0), (1, po1)):
                        nc.scalar.activation(
                            xvt[:, o, :], po, AF.Relu,
                            bias=b4af[:, o, bb : bb + 1],
                            scale=af[:, o, bb : bb + 1],
                        )
                    xqt = wp.tile([128, 2, TILE], F16, tag="xq")
                    xq1_pending.append((xqt, xvt, xvt))
                    pending.append((t, [xvt[:, 0, :], xvt[:, 1, :]],
                                    [xqt[:, 0, :], xqt[:, 1, :]]))
                for q in xq1_pending:
                    nc.vector.tensor_mul(*q)
                xq1_pending.clear()
                while pending or mv_c1_pending:
                    if mv_c1_pending:
                        emit_mv_c1()
                    else:
                        emit_mv_c0(pending.pop(0), pending.pop(0))
                    while epi_pending:
                        emit_epilogue()
                nc.sync.dma_start(
                    out=outv[:, 0, NPAIR // 2 : NPAIR, :],
                    in_=out2[0:1, NPAIR // 2 : NPAIR, :],
                )
                nc.sync.dma_start(
                    out=outv[:, 1, NPAIR // 2 : NPAIR, :],
                    in_=out2[32:33, NPAIR // 2 : NPAIR, :],
                )
    nc.compile()
    return nc


_NC_CACHE = {}


def _get_nc():
    if "nc" not in _NC_CACHE:
        _NC_CACHE["nc"] = _build_nc()
    return _NC_CACHE["nc"]


def _f16(x):
    return np.ascontiguousarray(x).astype(np.float16)


def kernel(**inputs) -> np.ndarray:
    object_feat = np.ascontiguousarray(np.asarray(inputs["object_feat"], np.float32))
    lang_feat = np.ascontiguousarray(np.asarray(inputs["lang_feat"], np.float32))
    lang_mask = np.asarray(inputs["lang_mask"])
    W3 = np.asarray(inputs["W3"], np.float32)
    W4 = np.asarray(inputs["W4"], np.float32)
    b3 = np.asarray(inputs["b3"], np.float32)
    b4 = np.asarray(inputs["b4"], np.float32)
    Wa = np.asarray(inputs["Wa"], np.float32)
    Ws = np.asarray(inputs["Ws"], np.float32)
    bs = np.asarray(inputs["bs"], np.float32)

    w3r = _f16(W3[0:256].reshape(2, 128, D).transpose(1, 0, 2).reshape(128, 2 * D))
    w3c2 = np.concatenate([W3[256:288, 0:128], W3[256:288, 128:256]], axis=0)
    w3cr = _f16(w3c2)
    w4r = _f16(W4.reshape(2, 128, D).transpose(1, 0, 2).reshape(128, 2 * D))
    csth = np.zeros((128, 4), np.float16)
    csth[:, 0:2] = _f16(Ws[:, 0].reshape(2, 128).T)
    csth[:, 2:4] = _f16(Wa[D:, 0].reshape(2, 128).T)

    in_maps = []
    for i in range(NCORES):
        sl = slice(i * BPC, (i + 1) * BPC)
        of = object_feat[sl]                                   # (BPC, N, C1)
        lf = lang_feat[sl]                                     # (BPC, M, D)
        xt = of.reshape(R, C1).T
        xt_dup = np.concatenate([xt, xt[256:288]], axis=0)     # (320, R)
        cstf = np.zeros((128, 29), np.float32)
        cstf[:, 0:2] = b3.reshape(2, 128).T
        cstf[:, 2:4] = b4.reshape(2, 128).T
        cstf[0, 4] = bs[0]
        cstf[0:M, 5:13] = lang_mask[sl].T.astype(np.float32)
        # b4 chunk-major, replicated across the BPC batches: [128, 2, 8]
        cstf[:, 13:29] = np.repeat(
            b4.reshape(2, 128).T[:, :, None], BPC, axis=2
        ).reshape(128, 2 * BPC)
        lt2 = lf.transpose(2, 0, 1).reshape(2, 128, BPC * M)
        in_maps.append(
            {
                "xt": _f16(xt_dup),
                "langm": _f16(lf.transpose(1, 0, 2)),
                "langt2": _f16(lt2.transpose(1, 0, 2).reshape(128, 2 * BPC * M)),
                "cstf": cstf,
                "csth": csth,
                "w3": w3r,
                "w3c": w3cr,
                "w4": w4r,
            }
        )

    nc = _get_nc()
    res = run_bass_kernel_spmd(nc, in_maps, core_ids=list(range(NCORES)))
    _NC_CACHE["last_results"] = res
    out = np.empty((B, 1, N), np.float32)
    for i in range(NCORES):
        out[i * BPC : (i + 1) * BPC, 0, :] = res.results[i]["out"].reshape(BPC, N)
    out += bs[0]  # final bias applied on host (constant add)
    return out
